# revision 29
# baseline (speedup 1.0000x reference)
"""Trainium2 Bass kernel for nn_KerasSaSentTensorflow (BiLSTM+CRF sentiment).

Strategy (data-parallel, per sharding hint):
  - The two large gate-preactivation ("x-part") matmuls of the BiLSTM are
    computed on the 8 NeuronCores, sharded across the batch*time rows:
      layer0: Xcat[8192,350] @ [Wx_fw0 | Wx_bw0][350,4096]
      layer1: out0[8192,1024] @ [Wx_fw1 | Wx_bw1][1024,4096]
    Matmuls run in bf16 (inputs + outputs) with fp32 PSUM accumulation:
    1 cycle/row on the PE array vs 4 for fp32, and half the HBM traffic.
  - The sequential time recurrences (h @ Wh per step, tiny work per step but
    strictly serial), CRF forward pass and the final head run on host.

The bass kernel is a K-accumulating tiled matmul, SPMD over 8 cores with the
M (row) dimension sharded; each core computes a [1024, 4096] slice.
"""
import contextlib
import ctypes
import os
import sys
import types

sys.path.insert(0, '/opt/trn_rl_repo')

import numpy as np
import ml_dtypes

BF16 = np.dtype(ml_dtypes.bfloat16)

B, T = 32, 256
WDIM, MDIM = 300, 50
HID, NCLASSES = 512, 3
N_CORES = 8
M_FULL = B * T            # 8192 rows (b-major: row = b*T + t)
M_LOC = M_FULL // N_CORES # 1024 rows per core
N_OUT = 4 * HID * 2       # 4096 = fw gates | bw gates
M_WAYS = 2                # row shard ways (cores 0-3 vs 4-7)
N_WAYS = 4                # gate-column shard ways (1024 cols per core)
NCH_LOC = N_OUT // N_WAYS // 512   # 512-col chunks per core
MB = 8                    # m-tiles per JIT-loaded at block

_CACHE = {}


def _install_ntff_hook():
    """Make bass_utils' trace=True path work under axon: synthesize
    antenv.axon_hooks and register the ctypes NTFF profile hook."""
    name = "antenv.axon_hooks"
    mod = sys.modules.get(name)
    if mod is None:
        mod = types.ModuleType(name)
        mod._hook = None
        mod.set_axon_ntff_profile_hook = lambda h, _m=mod: setattr(_m, "_hook", h)
        mod.get_axon_ntff_profile_hook = lambda _m=mod: _m._hook
        sys.modules[name] = mod
        try:
            import antenv
            antenv.axon_hooks = mod
        except ImportError:
            pass
    if mod._hook is not None:
        return
    lib = ctypes.CDLL("/opt/axon/libaxon_pjrt.so")
    if not hasattr(lib, "axon_start_nrt_profile"):
        return
    lib.axon_start_nrt_profile.argtypes = [ctypes.POINTER(ctypes.c_int64),
                                           ctypes.c_size_t]
    lib.axon_start_nrt_profile.restype = ctypes.c_int64
    lib.axon_stop_nrt_profile.argtypes = [ctypes.c_char_p]
    lib.axon_stop_nrt_profile.restype = ctypes.c_int64

    @contextlib.contextmanager
    def _hook(output_dir, device_ids):
        import jax
        jax.devices()
        if device_ids:
            ids = (ctypes.c_int64 * len(device_ids))(*device_ids)
            rc = lib.axon_start_nrt_profile(ids, len(device_ids))
        else:
            rc = lib.axon_start_nrt_profile(None, 0)
        if rc != 0:
            raise RuntimeError(f"axon_start_nrt_profile rc={rc}")
        try:
            yield
        finally:
            n = lib.axon_stop_nrt_profile(str(output_dir).encode())
            if n < 0:
                raise RuntimeError(f"axon_stop_nrt_profile rc={n}")

    mod.set_axon_ntff_profile_hook(_hook)


def _build_matmul_nc(K, MT):
    """out[MT*128, N_OUT] = at.T @ b  in bf16 with K contraction (K % 128 == 0)."""
    import concourse.bacc as bacc
    import concourse.mybir as mybir
    import concourse.tile as tile

    bf16 = mybir.dt.bfloat16
    f32 = mybir.dt.float32
    nc = bacc.Bacc("TRN2", target_bir_lowering=False, debug=False,
                   num_devices=N_CORES)
    KT = K // 128
    M_LC = MT * 128
    # Hybrid sharding: rows split 2 ways (cores 0-3 / 4-7), gate columns
    # split 4 ways (NCH_LOC chunks of 512 per core).  This keeps the global
    # 128-row tile count at its minimum (no per-core ceil-to-8 padding).
    # Layouts chosen for few, large DMAs (Sync-engine dispatch is ~0.7us per
    # dma_start):
    #   at: [K, M_LC] row-major; loaded per m-block just in time
    #   bm: host-permuted [128, NCH_LOC, KT, 512]; resident for whole kernel
    #   cm: device writes [128, NCH_LOC, MT, 512]; host permutes back
    at = nc.dram_tensor("at", [K, M_LC], bf16, kind="ExternalInput")
    bm = nc.dram_tensor("bm", [128, NCH_LOC * KT * 512], bf16,
                        kind="ExternalInput")
    cm = nc.dram_tensor("cm", [128, NCH_LOC * MT * 512], bf16,
                        kind="ExternalOutput")
    CHW = KT * 512      # bt chunk width (elements per partition)
    OGW = MT * 512      # out chunk width
    # Small first block (PE ramps in while DMA drip-feeds), then even split
    # of the rest — avoiding a tiny trailing block (costs ~3us of exposed
    # end-of-stream semaphore stall).
    first = min(4, MT)
    rest = MT - first
    n_rest = -(-rest // MB) if rest else 0
    plan = [first]
    if n_rest:
        base, rem = divmod(rest, n_rest)
        plan += [base + (1 if i < rem else 0) for i in range(n_rest)]
    n_blk = len(plan)
    with tile.TileContext(nc) as tc:
        with tc.tile_pool(name="wp", bufs=1) as wp, \
             tc.tile_pool(name="ab", bufs=2) as apool, \
             tc.tile_pool(name="op", bufs=2) as op, \
             tc.tile_pool(name="pp", bufs=8, space="PSUM") as pp:
            # Warmup matmuls on scratch SBUF: the PE clock (HAM gate) needs
            # ~3.4us of activity to ramp 1.2->2.4 GHz; burn that during the
            # initial DMA wait so real matmuls run at full clock.
            wsc = wp.tile([128, 512], bf16)
            nc.gpsimd.memset(wsc[:], 0)
            for _ in range(6):
                psw = pp.tile([128, 512], f32, tag="ps")
                nc.tensor.matmul(psw[:], wsc[:, :128], wsc[:], start=True,
                                 stop=True)
            # both bt chunks stay resident across all m-blocks
            bt_all = wp.tile([128, NCH_LOC * CHW], bf16)
            cast_engines = [nc.vector, nc.scalar]
            ci = 0
            m0 = 0
            for b, mts in enumerate(plan):
                BW = mts * 128
                at_b = apool.tile([128, KT * BW], bf16, tag="at")
                # Interleave this block's at k-tiles with the bt k-slices
                # (block 0 only) so the first groups can start early.
                for k in range(KT):
                    nc.sync.dma_start(
                        at_b[:, k * BW:(k + 1) * BW],
                        at.ap()[k * 128:(k + 1) * 128,
                                m0 * 128:m0 * 128 + BW])
                    if b == 0:
                        nc.sync.dma_start(
                            bt_all[:, k * 512:(k + 1) * 512],
                            bm.ap()[:, k * 512:(k + 1) * 512])
                if b == 0 and NCH_LOC > 1:
                    nc.sync.dma_start(bt_all[:, CHW:], bm.ap()[:, CHW:])
                last_blk = (b == n_blk - 1)
                for n in range(NCH_LOC):
                    bt = bt_all[:, n * CHW:(n + 1) * CHW]
                    ot = op.tile([128, mts * 512], bf16, tag="ot")
                    last = last_blk and (n == NCH_LOC - 1)
                    for m in range(mts):
                        ps = pp.tile([128, 512], f32, tag="ps")
                        for k in range(KT):
                            nc.tensor.matmul(
                                ps[:],
                                at_b[:, k * BW + m * 128:
                                        k * BW + (m + 1) * 128],
                                bt[:, k * 512:(k + 1) * 512],
                                start=(k == 0), stop=(k == KT - 1))
                        eng = cast_engines[ci % len(cast_engines)]
                        ci += 1
                        dst = ot[:, m * 512:(m + 1) * 512]
                        if eng is nc.scalar:
                            eng.copy(dst, ps[:])
                        else:
                            eng.tensor_copy(dst, ps[:])
                        if last:
                            # per-m output DMAs on the final piece: overlap
                            # DMA with the remaining casts (shorter tail)
                            nc.sync.dma_start(
                                cm.ap()[:, n * OGW + (m0 + m) * 512:
                                           n * OGW + (m0 + m + 1) * 512],
                                dst)
                    if not last:
                        nc.sync.dma_start(
                            cm.ap()[:, n * OGW + m0 * 512:
                                       n * OGW + (m0 + mts) * 512],
                            ot[:])
                m0 += mts
    nc.compile()
    return nc


def _device_matmul(a, bmat, sel=None):
    """a [M_FULL, K0] @ bmat [K0, N_OUT] on 8 cores (rows sharded). Pads K.

    If `sel` (sorted row indices) is given, only those rows are computed on
    device (the LSTM scan ignores t >= length rows); the rest return 0.
    """
    from concourse import bass_utils
    K0 = a.shape[1]
    K = ((K0 + 127) // 128) * 128
    rows = a if sel is None else a[sel]
    S = rows.shape[0]
    tiles = -(-S // 128)
    MT = max(1, -(-tiles // M_WAYS))            # ceil(ceil(S/128)/M_WAYS)
    M_LC = MT * 128
    M_TOT = M_LC * M_WAYS
    NQ = N_OUT // N_WAYS
    a_p = np.zeros((M_TOT, K), np.float32)
    a_p[:S, :K0] = rows
    b_p = np.zeros((K, N_OUT), np.float32)
    b_p[:K0, :] = bmat
    if (K, MT) not in _CACHE:
        _CACHE[(K, MT)] = _build_matmul_nc(K, MT)
    nc = _CACHE[(K, MT)]
    KT = K // 128
    at_full = np.ascontiguousarray(a_p.T).astype(BF16)  # [K, M_TOT]
    at_h = [np.ascontiguousarray(at_full[:, h * M_LC:(h + 1) * M_LC])
            for h in range(M_WAYS)]
    # quarter q: [K, NQ] -> [128, NCH_LOC, KT, 512]
    b_bf = b_p.astype(BF16)
    bm_q = [np.ascontiguousarray(
                b_bf[:, q * NQ:(q + 1) * NQ]
                .reshape(KT, 128, NCH_LOC, 512).transpose(1, 2, 0, 3)
            ).reshape(128, NCH_LOC * KT * 512) for q in range(N_WAYS)]
    in_maps = [{"at": at_h[c // N_WAYS], "bm": bm_q[c % N_WAYS]}
               for c in range(N_CORES)]
    trace = bool(os.environ.get("KERNEL_TRACE"))
    if trace:
        try:
            _install_ntff_hook()
        except Exception:
            trace = False
    res = bass_utils.run_bass_kernel_spmd(
        nc, in_maps, core_ids=list(range(N_CORES)), trace=trace)
    if res.exec_time_ns is not None:
        _device_matmul.exec_ns += res.exec_time_ns
    # per core: [128, NCH_LOC, MT, 512] -> [M_LC, NQ] block (row-half, col-q)
    out = np.empty((M_TOT, N_OUT), np.float32)
    for c in range(N_CORES):
        blk = (res.results[c]["cm"].reshape(128, NCH_LOC, MT, 512)
               .transpose(2, 0, 1, 3).reshape(M_LC, NQ))
        h, q = c // N_WAYS, c % N_WAYS
        out[h * M_LC:(h + 1) * M_LC, q * NQ:(q + 1) * NQ] = blk
    if sel is None:
        return out[:M_FULL]
    full = np.zeros((M_FULL, N_OUT), np.float32)
    full[sel] = out[:S]
    return full


_device_matmul.exec_ns = 0


def _sigmoid(x):
    return 1.0 / (1.0 + np.exp(-x))


def _lstm_scan(xpart, length, wh, bias, reverse):
    """TF LSTMCell recurrence given precomputed x-part of the gates.

    xpart: [B, T, 4H] = x_t @ Wx  (bias NOT included)
    wh:    [H, 4H] recurrent weights.  Masked-update dynamic_rnn semantics:
    bw direction == descending-t scan with the same (t < length) mask.
    """
    H = HID
    h = np.zeros((B, H), np.float32)
    c = np.zeros((B, H), np.float32)
    out = np.zeros((B, T, H), np.float32)
    wh = np.ascontiguousarray(wh, np.float32)
    bias = bias.astype(np.float32)
    trange = range(T - 1, -1, -1) if reverse else range(T)
    for t in trange:
        z = xpart[:, t] + h @ wh + bias
        i = z[:, 0:H]
        j = z[:, H:2 * H]
        f = z[:, 2 * H:3 * H]
        o = z[:, 3 * H:4 * H]
        c_new = _sigmoid(f + 1.0) * c + _sigmoid(i) * np.tanh(j)
        h_new = _sigmoid(o) * np.tanh(c_new)
        m = (t < length)[:, None]
        c = np.where(m, c_new, c)
        h = np.where(m, h_new, h)
        out[:, t] = np.where(m, h_new, 0.0)
    return out


def kernel(inputs_seq, masks, length, embedding, mask_embedding, transition,
           w_fw0, b_fw0, w_bw0, b_bw0, w_fw1, b_fw1, w_bw1, b_bw1,
           crf_w, crf_b, logits_w, logits_b):
    inputs_seq = np.asarray(inputs_seq)
    masks = np.asarray(masks)
    length = np.asarray(length).reshape(-1).astype(np.int64)
    embedding = np.asarray(embedding, np.float32)
    mask_embedding = np.asarray(mask_embedding, np.float32)
    transition = np.asarray(transition, np.float64)

    d0 = WDIM + MDIM
    # ---- input features (lookup = data prep) -------------------------------
    emb = embedding[inputs_seq]              # [B,T,300]
    memb = mask_embedding[masks]             # [B,T,50]
    xcat = np.concatenate([emb, memb], axis=-1).reshape(M_FULL, d0)

    # rows with t >= length[b] never contribute (scan zeroes/holds them), so
    # only compute x-parts for valid rows
    valid = (np.arange(T)[None, :] < length[:, None]).ravel()
    sel = np.flatnonzero(valid)
    if len(sel) == M_FULL:
        sel = None

    # ---- layer 0 x-part on device (8 cores, rows sharded) ------------------
    wx0 = np.concatenate([np.asarray(w_fw0, np.float32)[:d0],
                          np.asarray(w_bw0, np.float32)[:d0]], axis=1)
    xp0 = _device_matmul(xcat.astype(np.float32), wx0, sel)  # [8192, 4096]
    xp0 = xp0.reshape(B, T, 2, 4 * HID)

    fw0 = _lstm_scan(xp0[:, :, 0], length, np.asarray(w_fw0)[d0:],
                     np.asarray(b_fw0), reverse=False)
    bw0 = _lstm_scan(xp0[:, :, 1], length, np.asarray(w_bw0)[d0:],
                     np.asarray(b_bw0), reverse=True)
    out0 = np.concatenate([fw0, bw0], axis=-1)           # [B,T,1024]

    # ---- layer 1 x-part on device ------------------------------------------
    d1 = 2 * HID
    wx1 = np.concatenate([np.asarray(w_fw1, np.float32)[:d1],
                          np.asarray(w_bw1, np.float32)[:d1]], axis=1)
    xp1 = _device_matmul(out0.reshape(M_FULL, d1), wx1, sel)
    xp1 = xp1.reshape(B, T, 2, 4 * HID)

    fw1 = _lstm_scan(xp1[:, :, 0], length, np.asarray(w_fw1)[d1:],
                     np.asarray(b_fw1), reverse=False)
    bw1 = _lstm_scan(xp1[:, :, 1], length, np.asarray(w_bw1)[d1:],
                     np.asarray(b_bw1), reverse=True)
    out1 = np.concatenate([fw1, bw1], axis=-1)           # [B,T,1024]

    # ---- CRF forward probabilities over 2 tags -----------------------------
    e = out1 @ np.asarray(crf_w, np.float64) + np.asarray(crf_b, np.float64)
    alpha = e[:, 0]                                       # [B,2]
    probs = np.zeros((B, T, 2), np.float64)
    m0 = (length > 0)[:, None]
    probs[:, 0] = np.where(m0, _softmax(alpha), 0.0)
    for t in range(1, T):
        s = alpha[:, :, None] + transition[None]          # [B,2,2]
        mx = s.max(axis=1)
        new = mx + np.log(np.exp(s - mx[:, None]).sum(axis=1)) + e[:, t]
        m = (t < length)[:, None]
        alpha = np.where(m, new, alpha)
        probs[:, t] = np.where(m, _softmax(alpha), 0.0)

    # ---- head --------------------------------------------------------------
    p1 = probs[:, :, -1]                                  # [B,T]
    sv = np.einsum('bt,bth->bh', p1, out1)                # [B,1024]
    logits = sv @ np.asarray(logits_w, np.float64) + np.asarray(
        logits_b, np.float64)
    out = _softmax(logits).reshape(B, 1, NCLASSES)
    return out.astype(np.float32)


def _softmax(x):
    mx = x.max(axis=-1, keepdims=True)
    ex = np.exp(x - mx)
    return ex / ex.sum(axis=-1, keepdims=True)


# revision 30
# speedup vs baseline: 1.0283x; 1.0283x over previous
"""Trainium2 Bass kernel for nn_KerasSaSentTensorflow (BiLSTM+CRF sentiment).

Strategy (data-parallel, per sharding hint):
  - The two large gate-preactivation ("x-part") matmuls of the BiLSTM are
    computed on the 8 NeuronCores, sharded across the batch*time rows:
      layer0: Xcat[8192,350] @ [Wx_fw0 | Wx_bw0][350,4096]
      layer1: out0[8192,1024] @ [Wx_fw1 | Wx_bw1][1024,4096]
    Matmuls run in bf16 (inputs + outputs) with fp32 PSUM accumulation:
    1 cycle/row on the PE array vs 4 for fp32, and half the HBM traffic.
  - The sequential time recurrences (h @ Wh per step, tiny work per step but
    strictly serial), CRF forward pass and the final head run on host.

The bass kernel is a K-accumulating tiled matmul, SPMD over 8 cores with the
M (row) dimension sharded; each core computes a [1024, 4096] slice.
"""
import contextlib
import ctypes
import os
import sys
import types

sys.path.insert(0, '/opt/trn_rl_repo')

import numpy as np
import ml_dtypes

BF16 = np.dtype(ml_dtypes.bfloat16)

B, T = 32, 256
WDIM, MDIM = 300, 50
HID, NCLASSES = 512, 3
N_CORES = 8
M_FULL = B * T            # 8192 rows (b-major: row = b*T + t)
M_LOC = M_FULL // N_CORES # 1024 rows per core
N_OUT = 4 * HID * 2       # 4096 = fw gates | bw gates
M_WAYS = 2                # row shard ways (cores 0-3 vs 4-7)
N_WAYS = 4                # gate-column shard ways (1024 cols per core)
NCH_LOC = N_OUT // N_WAYS // 512   # 512-col chunks per core
MB = 8                    # m-tiles per JIT-loaded at block

_CACHE = {}


def _install_ntff_hook():
    """Make bass_utils' trace=True path work under axon: synthesize
    antenv.axon_hooks and register the ctypes NTFF profile hook."""
    name = "antenv.axon_hooks"
    mod = sys.modules.get(name)
    if mod is None:
        mod = types.ModuleType(name)
        mod._hook = None
        mod.set_axon_ntff_profile_hook = lambda h, _m=mod: setattr(_m, "_hook", h)
        mod.get_axon_ntff_profile_hook = lambda _m=mod: _m._hook
        sys.modules[name] = mod
        try:
            import antenv
            antenv.axon_hooks = mod
        except ImportError:
            pass
    if mod._hook is not None:
        return
    lib = ctypes.CDLL("/opt/axon/libaxon_pjrt.so")
    if not hasattr(lib, "axon_start_nrt_profile"):
        return
    lib.axon_start_nrt_profile.argtypes = [ctypes.POINTER(ctypes.c_int64),
                                           ctypes.c_size_t]
    lib.axon_start_nrt_profile.restype = ctypes.c_int64
    lib.axon_stop_nrt_profile.argtypes = [ctypes.c_char_p]
    lib.axon_stop_nrt_profile.restype = ctypes.c_int64

    @contextlib.contextmanager
    def _hook(output_dir, device_ids):
        import jax
        jax.devices()
        if device_ids:
            ids = (ctypes.c_int64 * len(device_ids))(*device_ids)
            rc = lib.axon_start_nrt_profile(ids, len(device_ids))
        else:
            rc = lib.axon_start_nrt_profile(None, 0)
        if rc != 0:
            raise RuntimeError(f"axon_start_nrt_profile rc={rc}")
        try:
            yield
        finally:
            n = lib.axon_stop_nrt_profile(str(output_dir).encode())
            if n < 0:
                raise RuntimeError(f"axon_stop_nrt_profile rc={n}")

    mod.set_axon_ntff_profile_hook(_hook)


def _build_matmul_nc(K, MT):
    """out[MT*128, N_OUT] = at.T @ b  in bf16 with K contraction (K % 128 == 0)."""
    import concourse.bacc as bacc
    import concourse.mybir as mybir
    import concourse.tile as tile

    bf16 = mybir.dt.bfloat16
    f32 = mybir.dt.float32
    nc = bacc.Bacc("TRN2", target_bir_lowering=False, debug=False,
                   num_devices=N_CORES)
    KT = K // 128
    M_LC = MT * 128
    # Hybrid sharding: rows split 2 ways (cores 0-3 / 4-7), gate columns
    # split 4 ways (NCH_LOC chunks of 512 per core).  This keeps the global
    # 128-row tile count at its minimum (no per-core ceil-to-8 padding).
    # Layouts chosen for few, large DMAs (Sync-engine dispatch is ~0.7us per
    # dma_start):
    #   at: [K, M_LC] row-major; loaded per m-block just in time
    #   bm: host-permuted [128, NCH_LOC, KT, 512]; resident for whole kernel
    #   cm: device writes [128, NCH_LOC, MT, 512]; host permutes back
    at = nc.dram_tensor("at", [K, M_LC], bf16, kind="ExternalInput")
    bm = nc.dram_tensor("bm", [128, NCH_LOC * KT * 512], bf16,
                        kind="ExternalInput")
    cm = nc.dram_tensor("cm", [128, NCH_LOC * MT * 512], bf16,
                        kind="ExternalOutput")
    CHW = KT * 512      # bt chunk width (elements per partition)
    OGW = MT * 512      # out chunk width
    # even split into ceil(MT/MB) blocks — avoids a tiny trailing block
    # (costs ~3us of exposed end-of-stream semaphore stall)
    n_blk = -(-MT // MB)
    base, rem = divmod(MT, n_blk)
    plan = [base + (1 if i < rem else 0) for i in range(n_blk)]
    with tile.TileContext(nc) as tc:
        with tc.tile_pool(name="wp", bufs=1) as wp, \
             tc.tile_pool(name="ab", bufs=2) as apool, \
             tc.tile_pool(name="op", bufs=2) as op, \
             tc.tile_pool(name="pp", bufs=8, space="PSUM") as pp:
            # Warmup matmuls on scratch SBUF: the PE clock (HAM gate) needs
            # ~3.4us of activity to ramp 1.2->2.4 GHz; burn that during the
            # initial DMA wait so real matmuls run at full clock.
            wsc = wp.tile([128, 512], bf16)
            nc.gpsimd.memset(wsc[:], 0)
            for _ in range(6):
                psw = pp.tile([128, 512], f32, tag="ps")
                nc.tensor.matmul(psw[:], wsc[:, :128], wsc[:], start=True,
                                 stop=True)
            # both bt chunks stay resident across all m-blocks
            bt_all = wp.tile([128, NCH_LOC * CHW], bf16)
            cast_engines = [nc.vector, nc.scalar]
            ci = 0
            m0 = 0
            for b, mts in enumerate(plan):
                BW = mts * 128
                at_b = apool.tile([128, KT * BW], bf16, tag="at")
                # Interleave this block's at k-tiles with the bt k-slices
                # (block 0 only) so the first groups can start early.
                for k in range(KT):
                    nc.sync.dma_start(
                        at_b[:, k * BW:(k + 1) * BW],
                        at.ap()[k * 128:(k + 1) * 128,
                                m0 * 128:m0 * 128 + BW])
                    if b == 0:
                        nc.sync.dma_start(
                            bt_all[:, k * 512:(k + 1) * 512],
                            bm.ap()[:, k * 512:(k + 1) * 512])
                if b == 0 and NCH_LOC > 1:
                    nc.sync.dma_start(bt_all[:, CHW:], bm.ap()[:, CHW:])
                last_blk = (b == n_blk - 1)
                for n in range(NCH_LOC):
                    bt = bt_all[:, n * CHW:(n + 1) * CHW]
                    ot = op.tile([128, mts * 512], bf16, tag="ot")
                    last = last_blk and (n == NCH_LOC - 1)
                    for m in range(mts):
                        ps = pp.tile([128, 512], f32, tag="ps")
                        for k in range(KT):
                            nc.tensor.matmul(
                                ps[:],
                                at_b[:, k * BW + m * 128:
                                        k * BW + (m + 1) * 128],
                                bt[:, k * 512:(k + 1) * 512],
                                start=(k == 0), stop=(k == KT - 1))
                        eng = cast_engines[ci % len(cast_engines)]
                        ci += 1
                        dst = ot[:, m * 512:(m + 1) * 512]
                        if eng is nc.scalar:
                            eng.copy(dst, ps[:])
                        else:
                            eng.tensor_copy(dst, ps[:])
                        if last:
                            # per-m output DMAs on the final piece: overlap
                            # DMA with the remaining casts (shorter tail)
                            nc.sync.dma_start(
                                cm.ap()[:, n * OGW + (m0 + m) * 512:
                                           n * OGW + (m0 + m + 1) * 512],
                                dst)
                    if not last:
                        nc.sync.dma_start(
                            cm.ap()[:, n * OGW + m0 * 512:
                                       n * OGW + (m0 + mts) * 512],
                            ot[:])
                m0 += mts
    nc.compile()
    return nc


def _device_matmul(a, bmat, sel=None):
    """a [M_FULL, K0] @ bmat [K0, N_OUT] on 8 cores (rows sharded). Pads K.

    If `sel` (sorted row indices) is given, only those rows are computed on
    device (the LSTM scan ignores t >= length rows); the rest return 0.
    """
    from concourse import bass_utils
    K0 = a.shape[1]
    K = ((K0 + 127) // 128) * 128
    rows = a if sel is None else a[sel]
    S = rows.shape[0]
    tiles = -(-S // 128)
    MT = max(1, -(-tiles // M_WAYS))            # ceil(ceil(S/128)/M_WAYS)
    M_LC = MT * 128
    M_TOT = M_LC * M_WAYS
    NQ = N_OUT // N_WAYS
    a_p = np.zeros((M_TOT, K), np.float32)
    a_p[:S, :K0] = rows
    b_p = np.zeros((K, N_OUT), np.float32)
    b_p[:K0, :] = bmat
    if (K, MT) not in _CACHE:
        _CACHE[(K, MT)] = _build_matmul_nc(K, MT)
    nc = _CACHE[(K, MT)]
    KT = K // 128
    at_full = np.ascontiguousarray(a_p.T).astype(BF16)  # [K, M_TOT]
    at_h = [np.ascontiguousarray(at_full[:, h * M_LC:(h + 1) * M_LC])
            for h in range(M_WAYS)]
    # quarter q: [K, NQ] -> [128, NCH_LOC, KT, 512]
    b_bf = b_p.astype(BF16)
    bm_q = [np.ascontiguousarray(
                b_bf[:, q * NQ:(q + 1) * NQ]
                .reshape(KT, 128, NCH_LOC, 512).transpose(1, 2, 0, 3)
            ).reshape(128, NCH_LOC * KT * 512) for q in range(N_WAYS)]
    in_maps = [{"at": at_h[c // N_WAYS], "bm": bm_q[c % N_WAYS]}
               for c in range(N_CORES)]
    trace = bool(os.environ.get("KERNEL_TRACE"))
    if trace:
        try:
            _install_ntff_hook()
        except Exception:
            trace = False
    res = bass_utils.run_bass_kernel_spmd(
        nc, in_maps, core_ids=list(range(N_CORES)), trace=trace)
    if res.exec_time_ns is not None:
        _device_matmul.exec_ns += res.exec_time_ns
    # per core: [128, NCH_LOC, MT, 512] -> [M_LC, NQ] block (row-half, col-q)
    out = np.empty((M_TOT, N_OUT), np.float32)
    for c in range(N_CORES):
        blk = (res.results[c]["cm"].reshape(128, NCH_LOC, MT, 512)
               .transpose(2, 0, 1, 3).reshape(M_LC, NQ))
        h, q = c // N_WAYS, c % N_WAYS
        out[h * M_LC:(h + 1) * M_LC, q * NQ:(q + 1) * NQ] = blk
    if sel is None:
        return out[:M_FULL]
    full = np.zeros((M_FULL, N_OUT), np.float32)
    full[sel] = out[:S]
    return full


_device_matmul.exec_ns = 0


def _sigmoid(x):
    return 1.0 / (1.0 + np.exp(-x))


def _lstm_scan(xpart, length, wh, bias, reverse):
    """TF LSTMCell recurrence given precomputed x-part of the gates.

    xpart: [B, T, 4H] = x_t @ Wx  (bias NOT included)
    wh:    [H, 4H] recurrent weights.  Masked-update dynamic_rnn semantics:
    bw direction == descending-t scan with the same (t < length) mask.
    """
    H = HID
    h = np.zeros((B, H), np.float32)
    c = np.zeros((B, H), np.float32)
    out = np.zeros((B, T, H), np.float32)
    wh = np.ascontiguousarray(wh, np.float32)
    bias = bias.astype(np.float32)
    trange = range(T - 1, -1, -1) if reverse else range(T)
    for t in trange:
        z = xpart[:, t] + h @ wh + bias
        i = z[:, 0:H]
        j = z[:, H:2 * H]
        f = z[:, 2 * H:3 * H]
        o = z[:, 3 * H:4 * H]
        c_new = _sigmoid(f + 1.0) * c + _sigmoid(i) * np.tanh(j)
        h_new = _sigmoid(o) * np.tanh(c_new)
        m = (t < length)[:, None]
        c = np.where(m, c_new, c)
        h = np.where(m, h_new, h)
        out[:, t] = np.where(m, h_new, 0.0)
    return out


def kernel(inputs_seq, masks, length, embedding, mask_embedding, transition,
           w_fw0, b_fw0, w_bw0, b_bw0, w_fw1, b_fw1, w_bw1, b_bw1,
           crf_w, crf_b, logits_w, logits_b):
    inputs_seq = np.asarray(inputs_seq)
    masks = np.asarray(masks)
    length = np.asarray(length).reshape(-1).astype(np.int64)
    embedding = np.asarray(embedding, np.float32)
    mask_embedding = np.asarray(mask_embedding, np.float32)
    transition = np.asarray(transition, np.float64)

    d0 = WDIM + MDIM
    # ---- input features (lookup = data prep) -------------------------------
    emb = embedding[inputs_seq]              # [B,T,300]
    memb = mask_embedding[masks]             # [B,T,50]
    xcat = np.concatenate([emb, memb], axis=-1).reshape(M_FULL, d0)

    # rows with t >= length[b] never contribute (scan zeroes/holds them), so
    # only compute x-parts for valid rows
    valid = (np.arange(T)[None, :] < length[:, None]).ravel()
    sel = np.flatnonzero(valid)
    if len(sel) == M_FULL:
        sel = None

    # ---- layer 0 x-part on device (8 cores, rows sharded) ------------------
    wx0 = np.concatenate([np.asarray(w_fw0, np.float32)[:d0],
                          np.asarray(w_bw0, np.float32)[:d0]], axis=1)
    xp0 = _device_matmul(xcat.astype(np.float32), wx0, sel)  # [8192, 4096]
    xp0 = xp0.reshape(B, T, 2, 4 * HID)

    fw0 = _lstm_scan(xp0[:, :, 0], length, np.asarray(w_fw0)[d0:],
                     np.asarray(b_fw0), reverse=False)
    bw0 = _lstm_scan(xp0[:, :, 1], length, np.asarray(w_bw0)[d0:],
                     np.asarray(b_bw0), reverse=True)
    out0 = np.concatenate([fw0, bw0], axis=-1)           # [B,T,1024]

    # ---- layer 1 x-part on device ------------------------------------------
    d1 = 2 * HID
    wx1 = np.concatenate([np.asarray(w_fw1, np.float32)[:d1],
                          np.asarray(w_bw1, np.float32)[:d1]], axis=1)
    xp1 = _device_matmul(out0.reshape(M_FULL, d1), wx1, sel)
    xp1 = xp1.reshape(B, T, 2, 4 * HID)

    fw1 = _lstm_scan(xp1[:, :, 0], length, np.asarray(w_fw1)[d1:],
                     np.asarray(b_fw1), reverse=False)
    bw1 = _lstm_scan(xp1[:, :, 1], length, np.asarray(w_bw1)[d1:],
                     np.asarray(b_bw1), reverse=True)
    out1 = np.concatenate([fw1, bw1], axis=-1)           # [B,T,1024]

    # ---- CRF forward probabilities over 2 tags -----------------------------
    e = out1 @ np.asarray(crf_w, np.float64) + np.asarray(crf_b, np.float64)
    alpha = e[:, 0]                                       # [B,2]
    probs = np.zeros((B, T, 2), np.float64)
    m0 = (length > 0)[:, None]
    probs[:, 0] = np.where(m0, _softmax(alpha), 0.0)
    for t in range(1, T):
        s = alpha[:, :, None] + transition[None]          # [B,2,2]
        mx = s.max(axis=1)
        new = mx + np.log(np.exp(s - mx[:, None]).sum(axis=1)) + e[:, t]
        m = (t < length)[:, None]
        alpha = np.where(m, new, alpha)
        probs[:, t] = np.where(m, _softmax(alpha), 0.0)

    # ---- head --------------------------------------------------------------
    p1 = probs[:, :, -1]                                  # [B,T]
    sv = np.einsum('bt,bth->bh', p1, out1)                # [B,1024]
    logits = sv @ np.asarray(logits_w, np.float64) + np.asarray(
        logits_b, np.float64)
    out = _softmax(logits).reshape(B, 1, NCLASSES)
    return out.astype(np.float32)


def _softmax(x):
    mx = x.max(axis=-1, keepdims=True)
    ex = np.exp(x - mx)
    return ex / ex.sum(axis=-1, keepdims=True)


# revision 36
# speedup vs baseline: 1.0687x; 1.0393x over previous
"""Trainium2 Bass kernel for nn_KerasSaSentTensorflow (BiLSTM+CRF sentiment).

Strategy (data-parallel, per sharding hint):
  - The two large gate-preactivation ("x-part") matmuls of the BiLSTM are
    computed on the 8 NeuronCores, sharded across the batch*time rows:
      layer0: Xcat[8192,350] @ [Wx_fw0 | Wx_bw0][350,4096]
      layer1: out0[8192,1024] @ [Wx_fw1 | Wx_bw1][1024,4096]
    Matmuls run in bf16 (inputs + outputs) with fp32 PSUM accumulation:
    1 cycle/row on the PE array vs 4 for fp32, and half the HBM traffic.
  - The sequential time recurrences (h @ Wh per step, tiny work per step but
    strictly serial), CRF forward pass and the final head run on host.

The bass kernel is a K-accumulating tiled matmul, SPMD over 8 cores with the
M (row) dimension sharded; each core computes a [1024, 4096] slice.
"""
import contextlib
import ctypes
import os
import sys
import types

sys.path.insert(0, '/opt/trn_rl_repo')

import numpy as np
import ml_dtypes

BF16 = np.dtype(ml_dtypes.bfloat16)
FP8 = np.dtype(ml_dtypes.float8_e4m3)

B, T = 32, 256
WDIM, MDIM = 300, 50
HID, NCLASSES = 512, 3
N_CORES = 8
M_FULL = B * T            # 8192 rows (b-major: row = b*T + t)
M_LOC = M_FULL // N_CORES # 1024 rows per core
N_OUT = 4 * HID * 2       # 4096 = fw gates | bw gates
M_WAYS = 2                # row shard ways (cores 0-3 vs 4-7)
N_WAYS = 4                # gate-column shard ways (1024 cols per core)
NCH_LOC = N_OUT // N_WAYS // 512   # 512-col chunks per core
MB = 8                    # m-tiles per JIT-loaded at block

_CACHE = {}


def _install_ntff_hook():
    """Make bass_utils' trace=True path work under axon: synthesize
    antenv.axon_hooks and register the ctypes NTFF profile hook."""
    name = "antenv.axon_hooks"
    mod = sys.modules.get(name)
    if mod is None:
        mod = types.ModuleType(name)
        mod._hook = None
        mod.set_axon_ntff_profile_hook = lambda h, _m=mod: setattr(_m, "_hook", h)
        mod.get_axon_ntff_profile_hook = lambda _m=mod: _m._hook
        sys.modules[name] = mod
        try:
            import antenv
            antenv.axon_hooks = mod
        except ImportError:
            pass
    if mod._hook is not None:
        return
    lib = ctypes.CDLL("/opt/axon/libaxon_pjrt.so")
    if not hasattr(lib, "axon_start_nrt_profile"):
        return
    lib.axon_start_nrt_profile.argtypes = [ctypes.POINTER(ctypes.c_int64),
                                           ctypes.c_size_t]
    lib.axon_start_nrt_profile.restype = ctypes.c_int64
    lib.axon_stop_nrt_profile.argtypes = [ctypes.c_char_p]
    lib.axon_stop_nrt_profile.restype = ctypes.c_int64

    @contextlib.contextmanager
    def _hook(output_dir, device_ids):
        import jax
        jax.devices()
        if device_ids:
            ids = (ctypes.c_int64 * len(device_ids))(*device_ids)
            rc = lib.axon_start_nrt_profile(ids, len(device_ids))
        else:
            rc = lib.axon_start_nrt_profile(None, 0)
        if rc != 0:
            raise RuntimeError(f"axon_start_nrt_profile rc={rc}")
        try:
            yield
        finally:
            n = lib.axon_stop_nrt_profile(str(output_dir).encode())
            if n < 0:
                raise RuntimeError(f"axon_stop_nrt_profile rc={n}")

    mod.set_axon_ntff_profile_hook(_hook)


def _build_matmul_nc(K, MT, K8=0):
    """out[MT*128, N_OUT/4] = at.T @ b, K contraction (K % 128 == 0).

    If K8 > 0 (multiple of 256), the first K8 contraction rows run as
    fp8e4+DoubleRow pair-matmuls (2 k-tiles per instruction, ~1.44x faster)
    and the remaining K-K8 rows in bf16; the fp8 fraction is kept small to
    bound the accuracy loss.
    """
    import concourse.bacc as bacc
    import concourse.mybir as mybir
    import concourse.tile as tile

    bf16 = mybir.dt.bfloat16
    fp8 = mybir.dt.float8e4
    f32 = mybir.dt.float32
    nc = bacc.Bacc("TRN2", target_bir_lowering=False, debug=False,
                   num_devices=N_CORES)
    KT = (K - K8) // 128     # bf16 k-tiles
    KP = K8 // 256           # fp8 DoubleRow pair-instructions
    M_LC = MT * 128
    # Hybrid sharding: rows split 2 ways (cores 0-3 / 4-7), gate columns
    # split 4 ways (NCH_LOC chunks of 512 per core).  This keeps the global
    # 128-row tile count at its minimum (no per-core ceil-to-8 padding).
    # Layouts chosen for few, large DMAs (Sync-engine dispatch is ~0.7us per
    # dma_start):
    #   at: [K-K8, M_LC] row-major; loaded per m-block just in time
    #   bm: host-permuted [128, NCH_LOC, KT, 512]; resident for whole kernel
    #   cm: device writes [128, NCH_LOC, MT, 512]; host permutes back
    at = nc.dram_tensor("at", [K - K8, M_LC], bf16, kind="ExternalInput")
    bm = nc.dram_tensor("bm", [128, NCH_LOC * KT * 512], bf16,
                        kind="ExternalInput")
    cm = nc.dram_tensor("cm", [128, NCH_LOC * MT * 512], bf16,
                        kind="ExternalOutput")
    if K8:
        # fp8 part: at8 rows are k-subtile-major ([KP, 2, 128, M_LC] in dram);
        # bm8 host layout [128, NCH_LOC, KP, 2, 512]
        at8 = nc.dram_tensor("at8", [K8, M_LC], fp8, kind="ExternalInput")
        bm8 = nc.dram_tensor("bm8", [128, NCH_LOC * KP * 2 * 512], fp8,
                             kind="ExternalInput")
    CHW = KT * 512      # bt chunk width (elements per partition)
    OGW = MT * 512      # out chunk width
    # even split into ceil(MT/MB) blocks — avoids a tiny trailing block
    # (costs ~3us of exposed end-of-stream semaphore stall)
    n_blk = -(-MT // MB)
    base, rem = divmod(MT, n_blk)
    plan = [base + (1 if i < rem else 0) for i in range(n_blk)]
    with tile.TileContext(nc) as tc:
        with tc.tile_pool(name="wp", bufs=1) as wp, \
             tc.tile_pool(name="ab", bufs=2) as apool, \
             tc.tile_pool(name="op", bufs=2) as op, \
             tc.tile_pool(name="pp", bufs=8, space="PSUM") as pp:
            # Warmup matmuls on scratch SBUF: the PE clock (HAM gate) needs
            # ~3.4us of activity to ramp 1.2->2.4 GHz; burn that during the
            # initial DMA wait so real matmuls run at full clock.
            wsc = wp.tile([128, 512], bf16)
            nc.gpsimd.memset(wsc[:], 0)
            for _ in range(6):
                psw = pp.tile([128, 512], f32, tag="ps")
                nc.tensor.matmul(psw[:], wsc[:, :128], wsc[:], start=True,
                                 stop=True)
            # both bt chunks stay resident across all m-blocks
            bt_all = wp.tile([128, NCH_LOC * CHW], bf16)
            if K8:
                # fp8 weights resident: [128, NCH_LOC*KP*2, 512]
                bt8_all = wp.tile([128, NCH_LOC * KP * 2, 512], fp8)
                for i in range(NCH_LOC * KP * 2):
                    nc.sync.dma_start(
                        bt8_all[:, i, :],
                        bm8.ap()[:, i * 512:(i + 1) * 512])
            cast_engines = [nc.vector, nc.scalar]
            ci = 0
            m0 = 0
            for b, mts in enumerate(plan):
                BW = mts * 128
                at_b = apool.tile([128, KT * BW], bf16, tag="at")
                if K8:
                    at8_b = apool.tile([128, KP * 2, BW], fp8, tag="at8")
                    for s in range(KP * 2):
                        nc.sync.dma_start(
                            at8_b[:, s, :],
                            at8.ap()[s * 128:(s + 1) * 128,
                                     m0 * 128:m0 * 128 + BW])
                # Interleave this block's at k-tiles with the bt k-slices
                # (block 0 only) so the first groups can start early.
                for k in range(KT):
                    nc.sync.dma_start(
                        at_b[:, k * BW:(k + 1) * BW],
                        at.ap()[k * 128:(k + 1) * 128,
                                m0 * 128:m0 * 128 + BW])
                    if b == 0:
                        nc.sync.dma_start(
                            bt_all[:, k * 512:(k + 1) * 512],
                            bm.ap()[:, k * 512:(k + 1) * 512])
                if b == 0 and NCH_LOC > 1:
                    nc.sync.dma_start(bt_all[:, CHW:], bm.ap()[:, CHW:])
                last_blk = (b == n_blk - 1)
                for n in range(NCH_LOC):
                    bt = bt_all[:, n * CHW:(n + 1) * CHW]
                    ot = op.tile([128, mts * 512], bf16, tag="ot")
                    last = last_blk and (n == NCH_LOC - 1)
                    for m in range(mts):
                        ps = pp.tile([128, 512], f32, tag="ps")
                        for j in range(KP):
                            nc.tensor.matmul(
                                ps[:],
                                at8_b[:, 2 * j:2 * j + 2,
                                      m * 128:(m + 1) * 128],
                                bt8_all[:, n * KP * 2 + 2 * j:
                                           n * KP * 2 + 2 * j + 2, :],
                                start=(j == 0), stop=False,
                                perf_mode=mybir.MatmulPerfMode.DoubleRow)
                        for k in range(KT):
                            nc.tensor.matmul(
                                ps[:],
                                at_b[:, k * BW + m * 128:
                                        k * BW + (m + 1) * 128],
                                bt[:, k * 512:(k + 1) * 512],
                                start=(k == 0 and not K8),
                                stop=(k == KT - 1))
                        eng = cast_engines[ci % len(cast_engines)]
                        ci += 1
                        dst = ot[:, m * 512:(m + 1) * 512]
                        if eng is nc.scalar:
                            eng.copy(dst, ps[:])
                        else:
                            eng.tensor_copy(dst, ps[:])
                        if last:
                            # per-m output DMAs on the final piece: overlap
                            # DMA with the remaining casts (shorter tail)
                            nc.sync.dma_start(
                                cm.ap()[:, n * OGW + (m0 + m) * 512:
                                           n * OGW + (m0 + m + 1) * 512],
                                dst)
                    if not last:
                        nc.sync.dma_start(
                            cm.ap()[:, n * OGW + m0 * 512:
                                       n * OGW + (m0 + mts) * 512],
                            ot[:])
                m0 += mts
    nc.compile()
    return nc


def _device_matmul(a, bmat, sel=None):
    """a [M_FULL, K0] @ bmat [K0, N_OUT] on 8 cores (rows sharded). Pads K.

    If `sel` (sorted row indices) is given, only those rows are computed on
    device (the LSTM scan ignores t >= length rows); the rest return 0.
    """
    from concourse import bass_utils
    K0 = a.shape[1]
    K = ((K0 + 127) // 128) * 128
    rows = a if sel is None else a[sel]
    S = rows.shape[0]
    tiles = -(-S // 128)
    MT = max(1, -(-tiles // M_WAYS))            # ceil(ceil(S/128)/M_WAYS)
    M_LC = MT * 128
    M_TOT = M_LC * M_WAYS
    NQ = N_OUT // N_WAYS
    a_p = np.zeros((M_TOT, K), np.float32)
    a_p[:S, :K0] = rows
    b_p = np.zeros((K, N_OUT), np.float32)
    b_p[:K0, :] = bmat
    # quarter of layer-1's K runs in fp8+DoubleRow (1.44x); simulated max
    # rel err 1.1e-2 vs the 2e-2 gate.  L0 stays pure bf16.
    K8 = 256 if K == 1024 else 0
    KP = K8 // 256
    if (K, MT) not in _CACHE:
        _CACHE[(K, MT)] = _build_matmul_nc(K, MT, K8)
    nc = _CACHE[(K, MT)]
    KT = (K - K8) // 128
    at_full = np.ascontiguousarray(a_p.T)               # [K, M_TOT] f32
    at16 = at_full[K8:].astype(BF16)
    at_h = [np.ascontiguousarray(at16[:, h * M_LC:(h + 1) * M_LC])
            for h in range(M_WAYS)]
    # quarter q: [K-K8, NQ] -> [128, NCH_LOC, KT, 512]
    b_bf = b_p[K8:].astype(BF16)
    bm_q = [np.ascontiguousarray(
                b_bf[:, q * NQ:(q + 1) * NQ]
                .reshape(KT, 128, NCH_LOC, 512).transpose(1, 2, 0, 3)
            ).reshape(128, NCH_LOC * KT * 512) for q in range(N_WAYS)]
    in_maps = [{"at": at_h[c // N_WAYS], "bm": bm_q[c % N_WAYS]}
               for c in range(N_CORES)]
    if K8:
        at8_full = at_full[:K8].astype(FP8)
        at8_h = [np.ascontiguousarray(at8_full[:, h * M_LC:(h + 1) * M_LC])
                 for h in range(M_WAYS)]
        b8 = b_p[:K8].astype(FP8)
        # [K8, NQ] -> [128, NCH_LOC, KP, 2, 512]
        bm8_q = [np.ascontiguousarray(
                     b8[:, q * NQ:(q + 1) * NQ]
                     .reshape(KP, 2, 128, NCH_LOC, 512)
                     .transpose(2, 3, 0, 1, 4)
                 ).reshape(128, NCH_LOC * KP * 2 * 512)
                 for q in range(N_WAYS)]
        for c in range(N_CORES):
            in_maps[c]["at8"] = at8_h[c // N_WAYS]
            in_maps[c]["bm8"] = bm8_q[c % N_WAYS]
    trace = bool(os.environ.get("KERNEL_TRACE"))
    if trace:
        try:
            _install_ntff_hook()
        except Exception:
            trace = False
    res = bass_utils.run_bass_kernel_spmd(
        nc, in_maps, core_ids=list(range(N_CORES)), trace=trace)
    if res.exec_time_ns is not None:
        _device_matmul.exec_ns += res.exec_time_ns
    # per core: [128, NCH_LOC, MT, 512] -> [M_LC, NQ] block (row-half, col-q)
    out = np.empty((M_TOT, N_OUT), np.float32)
    for c in range(N_CORES):
        blk = (res.results[c]["cm"].reshape(128, NCH_LOC, MT, 512)
               .transpose(2, 0, 1, 3).reshape(M_LC, NQ))
        h, q = c // N_WAYS, c % N_WAYS
        out[h * M_LC:(h + 1) * M_LC, q * NQ:(q + 1) * NQ] = blk
    if sel is None:
        return out[:M_FULL]
    full = np.zeros((M_FULL, N_OUT), np.float32)
    full[sel] = out[:S]
    return full


_device_matmul.exec_ns = 0


def _sigmoid(x):
    return 1.0 / (1.0 + np.exp(-x))


def _lstm_scan(xpart, length, wh, bias, reverse):
    """TF LSTMCell recurrence given precomputed x-part of the gates.

    xpart: [B, T, 4H] = x_t @ Wx  (bias NOT included)
    wh:    [H, 4H] recurrent weights.  Masked-update dynamic_rnn semantics:
    bw direction == descending-t scan with the same (t < length) mask.
    """
    H = HID
    h = np.zeros((B, H), np.float32)
    c = np.zeros((B, H), np.float32)
    out = np.zeros((B, T, H), np.float32)
    wh = np.ascontiguousarray(wh, np.float32)
    bias = bias.astype(np.float32)
    trange = range(T - 1, -1, -1) if reverse else range(T)
    for t in trange:
        z = xpart[:, t] + h @ wh + bias
        i = z[:, 0:H]
        j = z[:, H:2 * H]
        f = z[:, 2 * H:3 * H]
        o = z[:, 3 * H:4 * H]
        c_new = _sigmoid(f + 1.0) * c + _sigmoid(i) * np.tanh(j)
        h_new = _sigmoid(o) * np.tanh(c_new)
        m = (t < length)[:, None]
        c = np.where(m, c_new, c)
        h = np.where(m, h_new, h)
        out[:, t] = np.where(m, h_new, 0.0)
    return out


def kernel(inputs_seq, masks, length, embedding, mask_embedding, transition,
           w_fw0, b_fw0, w_bw0, b_bw0, w_fw1, b_fw1, w_bw1, b_bw1,
           crf_w, crf_b, logits_w, logits_b):
    inputs_seq = np.asarray(inputs_seq)
    masks = np.asarray(masks)
    length = np.asarray(length).reshape(-1).astype(np.int64)
    embedding = np.asarray(embedding, np.float32)
    mask_embedding = np.asarray(mask_embedding, np.float32)
    transition = np.asarray(transition, np.float64)

    d0 = WDIM + MDIM
    # ---- input features (lookup = data prep) -------------------------------
    emb = embedding[inputs_seq]              # [B,T,300]
    memb = mask_embedding[masks]             # [B,T,50]
    xcat = np.concatenate([emb, memb], axis=-1).reshape(M_FULL, d0)

    # rows with t >= length[b] never contribute (scan zeroes/holds them), so
    # only compute x-parts for valid rows
    valid = (np.arange(T)[None, :] < length[:, None]).ravel()
    sel = np.flatnonzero(valid)
    if len(sel) == M_FULL:
        sel = None

    # ---- layer 0 x-part on device (8 cores, rows sharded) ------------------
    wx0 = np.concatenate([np.asarray(w_fw0, np.float32)[:d0],
                          np.asarray(w_bw0, np.float32)[:d0]], axis=1)
    xp0 = _device_matmul(xcat.astype(np.float32), wx0, sel)  # [8192, 4096]
    xp0 = xp0.reshape(B, T, 2, 4 * HID)

    fw0 = _lstm_scan(xp0[:, :, 0], length, np.asarray(w_fw0)[d0:],
                     np.asarray(b_fw0), reverse=False)
    bw0 = _lstm_scan(xp0[:, :, 1], length, np.asarray(w_bw0)[d0:],
                     np.asarray(b_bw0), reverse=True)
    out0 = np.concatenate([fw0, bw0], axis=-1)           # [B,T,1024]

    # ---- layer 1 x-part on device ------------------------------------------
    d1 = 2 * HID
    wx1 = np.concatenate([np.asarray(w_fw1, np.float32)[:d1],
                          np.asarray(w_bw1, np.float32)[:d1]], axis=1)
    xp1 = _device_matmul(out0.reshape(M_FULL, d1), wx1, sel)
    xp1 = xp1.reshape(B, T, 2, 4 * HID)

    fw1 = _lstm_scan(xp1[:, :, 0], length, np.asarray(w_fw1)[d1:],
                     np.asarray(b_fw1), reverse=False)
    bw1 = _lstm_scan(xp1[:, :, 1], length, np.asarray(w_bw1)[d1:],
                     np.asarray(b_bw1), reverse=True)
    out1 = np.concatenate([fw1, bw1], axis=-1)           # [B,T,1024]

    # ---- CRF forward probabilities over 2 tags -----------------------------
    e = out1 @ np.asarray(crf_w, np.float64) + np.asarray(crf_b, np.float64)
    alpha = e[:, 0]                                       # [B,2]
    probs = np.zeros((B, T, 2), np.float64)
    m0 = (length > 0)[:, None]
    probs[:, 0] = np.where(m0, _softmax(alpha), 0.0)
    for t in range(1, T):
        s = alpha[:, :, None] + transition[None]          # [B,2,2]
        mx = s.max(axis=1)
        new = mx + np.log(np.exp(s - mx[:, None]).sum(axis=1)) + e[:, t]
        m = (t < length)[:, None]
        alpha = np.where(m, new, alpha)
        probs[:, t] = np.where(m, _softmax(alpha), 0.0)

    # ---- head --------------------------------------------------------------
    p1 = probs[:, :, -1]                                  # [B,T]
    sv = np.einsum('bt,bth->bh', p1, out1)                # [B,1024]
    logits = sv @ np.asarray(logits_w, np.float64) + np.asarray(
        logits_b, np.float64)
    out = _softmax(logits).reshape(B, 1, NCLASSES)
    return out.astype(np.float32)


def _softmax(x):
    mx = x.max(axis=-1, keepdims=True)
    ex = np.exp(x - mx)
    return ex / ex.sum(axis=-1, keepdims=True)


# revision 37
# speedup vs baseline: 1.0807x; 1.0112x over previous
"""Trainium2 Bass kernel for nn_KerasSaSentTensorflow (BiLSTM+CRF sentiment).

Strategy (data-parallel, per sharding hint):
  - The two large gate-preactivation ("x-part") matmuls of the BiLSTM are
    computed on the 8 NeuronCores, sharded across the batch*time rows:
      layer0: Xcat[8192,350] @ [Wx_fw0 | Wx_bw0][350,4096]
      layer1: out0[8192,1024] @ [Wx_fw1 | Wx_bw1][1024,4096]
    Matmuls run in bf16 (inputs + outputs) with fp32 PSUM accumulation:
    1 cycle/row on the PE array vs 4 for fp32, and half the HBM traffic.
  - The sequential time recurrences (h @ Wh per step, tiny work per step but
    strictly serial), CRF forward pass and the final head run on host.

The bass kernel is a K-accumulating tiled matmul, SPMD over 8 cores with the
M (row) dimension sharded; each core computes a [1024, 4096] slice.
"""
import contextlib
import ctypes
import os
import sys
import types

sys.path.insert(0, '/opt/trn_rl_repo')

import numpy as np
import ml_dtypes

BF16 = np.dtype(ml_dtypes.bfloat16)
FP8 = np.dtype(ml_dtypes.float8_e4m3)

B, T = 32, 256
WDIM, MDIM = 300, 50
HID, NCLASSES = 512, 3
N_CORES = 8
M_FULL = B * T            # 8192 rows (b-major: row = b*T + t)
M_LOC = M_FULL // N_CORES # 1024 rows per core
N_OUT = 4 * HID * 2       # 4096 = fw gates | bw gates
M_WAYS = 2                # row shard ways (cores 0-3 vs 4-7)
N_WAYS = 4                # gate-column shard ways (1024 cols per core)
NCH_LOC = N_OUT // N_WAYS // 512   # 512-col chunks per core
MB = 8                    # m-tiles per JIT-loaded at block

_CACHE = {}


def _install_ntff_hook():
    """Make bass_utils' trace=True path work under axon: synthesize
    antenv.axon_hooks and register the ctypes NTFF profile hook."""
    name = "antenv.axon_hooks"
    mod = sys.modules.get(name)
    if mod is None:
        mod = types.ModuleType(name)
        mod._hook = None
        mod.set_axon_ntff_profile_hook = lambda h, _m=mod: setattr(_m, "_hook", h)
        mod.get_axon_ntff_profile_hook = lambda _m=mod: _m._hook
        sys.modules[name] = mod
        try:
            import antenv
            antenv.axon_hooks = mod
        except ImportError:
            pass
    if mod._hook is not None:
        return
    lib = ctypes.CDLL("/opt/axon/libaxon_pjrt.so")
    if not hasattr(lib, "axon_start_nrt_profile"):
        return
    lib.axon_start_nrt_profile.argtypes = [ctypes.POINTER(ctypes.c_int64),
                                           ctypes.c_size_t]
    lib.axon_start_nrt_profile.restype = ctypes.c_int64
    lib.axon_stop_nrt_profile.argtypes = [ctypes.c_char_p]
    lib.axon_stop_nrt_profile.restype = ctypes.c_int64

    @contextlib.contextmanager
    def _hook(output_dir, device_ids):
        import jax
        jax.devices()
        if device_ids:
            ids = (ctypes.c_int64 * len(device_ids))(*device_ids)
            rc = lib.axon_start_nrt_profile(ids, len(device_ids))
        else:
            rc = lib.axon_start_nrt_profile(None, 0)
        if rc != 0:
            raise RuntimeError(f"axon_start_nrt_profile rc={rc}")
        try:
            yield
        finally:
            n = lib.axon_stop_nrt_profile(str(output_dir).encode())
            if n < 0:
                raise RuntimeError(f"axon_stop_nrt_profile rc={n}")

    mod.set_axon_ntff_profile_hook(_hook)


def _build_matmul_nc(K, MT, K8=0):
    """out[MT*128, N_OUT/4] = at.T @ b, K contraction (K % 128 == 0).

    If K8 > 0 (multiple of 256), the first K8 contraction rows run as
    fp8e4+DoubleRow pair-matmuls (2 k-tiles per instruction, ~1.44x faster)
    and the remaining K-K8 rows in bf16; the fp8 fraction is kept small to
    bound the accuracy loss.
    """
    import concourse.bacc as bacc
    import concourse.mybir as mybir
    import concourse.tile as tile

    bf16 = mybir.dt.bfloat16
    fp8 = mybir.dt.float8e4
    f32 = mybir.dt.float32
    nc = bacc.Bacc("TRN2", target_bir_lowering=False, debug=False,
                   num_devices=N_CORES)
    KT = (K - K8) // 128     # bf16 k-tiles
    KP = K8 // 256           # fp8 DoubleRow pair-instructions
    M_LC = MT * 128
    # Hybrid sharding: rows split 2 ways (cores 0-3 / 4-7), gate columns
    # split 4 ways (NCH_LOC chunks of 512 per core).  This keeps the global
    # 128-row tile count at its minimum (no per-core ceil-to-8 padding).
    # Layouts chosen for few, large DMAs (Sync-engine dispatch is ~0.7us per
    # dma_start):
    #   at: [K-K8, M_LC] row-major; loaded per m-block just in time
    #   bm: host-permuted [128, NCH_LOC, KT, 512]; resident for whole kernel
    #   cm: device writes [128, NCH_LOC, MT, 512]; host permutes back
    at = nc.dram_tensor("at", [K - K8, M_LC], bf16, kind="ExternalInput")
    bm = nc.dram_tensor("bm", [128, NCH_LOC * KT * 512], bf16,
                        kind="ExternalInput")
    cm = nc.dram_tensor("cm", [128, NCH_LOC * MT * 512], bf16,
                        kind="ExternalOutput")
    if K8:
        # fp8 part: at8 rows are k-subtile-major ([KP, 2, 128, M_LC] in dram);
        # bm8 host layout [128, NCH_LOC, KP, 2, 512]
        at8 = nc.dram_tensor("at8", [K8, M_LC], fp8, kind="ExternalInput")
        bm8 = nc.dram_tensor("bm8", [128, NCH_LOC * KP * 2 * 512], fp8,
                             kind="ExternalInput")
    CHW = KT * 512      # bt chunk width (elements per partition)
    OGW = MT * 512      # out chunk width
    # even split into ceil(MT/MB) blocks — avoids a tiny trailing block
    # (costs ~3us of exposed end-of-stream semaphore stall)
    n_blk = -(-MT // MB)
    base, rem = divmod(MT, n_blk)
    plan = [base + (1 if i < rem else 0) for i in range(n_blk)]
    with tile.TileContext(nc) as tc:
        with tc.tile_pool(name="wp", bufs=1) as wp, \
             tc.tile_pool(name="ab", bufs=2) as apool, \
             tc.tile_pool(name="op", bufs=2) as op, \
             tc.tile_pool(name="pp", bufs=8, space="PSUM") as pp:
            # Warmup matmuls on scratch SBUF: the PE clock (HAM gate) needs
            # ~3.4us of activity to ramp 1.2->2.4 GHz; burn that during the
            # initial DMA wait so real matmuls run at full clock.
            wsc = wp.tile([128, 512], bf16)
            nc.gpsimd.memset(wsc[:], 0)
            for _ in range(6):
                psw = pp.tile([128, 512], f32, tag="ps")
                nc.tensor.matmul(psw[:], wsc[:, :128], wsc[:], start=True,
                                 stop=True)
            # both bt chunks stay resident across all m-blocks
            bt_all = wp.tile([128, NCH_LOC * CHW], bf16)
            if K8:
                # fp8 weights resident: [128, NCH_LOC*KP*2, 512]
                bt8_all = wp.tile([128, NCH_LOC * KP * 2, 512], fp8)
                for i in range(NCH_LOC * KP * 2):
                    nc.sync.dma_start(
                        bt8_all[:, i, :],
                        bm8.ap()[:, i * 512:(i + 1) * 512])
            cast_engines = [nc.vector, nc.scalar]
            ci = 0
            m0 = 0
            for b, mts in enumerate(plan):
                BW = mts * 128
                at_b = apool.tile([128, KT * BW], bf16, tag="at")
                if K8:
                    at8_b = apool.tile([128, KP * 2, BW], fp8, tag="at8")
                    for s in range(KP * 2):
                        nc.sync.dma_start(
                            at8_b[:, s, :],
                            at8.ap()[s * 128:(s + 1) * 128,
                                     m0 * 128:m0 * 128 + BW])
                # Interleave this block's at k-tiles with the bt k-slices
                # (block 0 only) so the first groups can start early.
                for k in range(KT):
                    nc.sync.dma_start(
                        at_b[:, k * BW:(k + 1) * BW],
                        at.ap()[k * 128:(k + 1) * 128,
                                m0 * 128:m0 * 128 + BW])
                    if b == 0:
                        nc.sync.dma_start(
                            bt_all[:, k * 512:(k + 1) * 512],
                            bm.ap()[:, k * 512:(k + 1) * 512])
                if b == 0 and NCH_LOC > 1:
                    nc.sync.dma_start(bt_all[:, CHW:], bm.ap()[:, CHW:])
                last_blk = (b == n_blk - 1)
                for n in range(NCH_LOC):
                    bt = bt_all[:, n * CHW:(n + 1) * CHW]
                    ot = op.tile([128, mts * 512], bf16, tag="ot")
                    last = last_blk and (n == NCH_LOC - 1)
                    for m in range(mts):
                        ps = pp.tile([128, 512], f32, tag="ps")
                        for j in range(KP):
                            nc.tensor.matmul(
                                ps[:],
                                at8_b[:, 2 * j:2 * j + 2,
                                      m * 128:(m + 1) * 128],
                                bt8_all[:, n * KP * 2 + 2 * j:
                                           n * KP * 2 + 2 * j + 2, :],
                                start=(j == 0), stop=False,
                                perf_mode=mybir.MatmulPerfMode.DoubleRow)
                        for k in range(KT):
                            nc.tensor.matmul(
                                ps[:],
                                at_b[:, k * BW + m * 128:
                                        k * BW + (m + 1) * 128],
                                bt[:, k * 512:(k + 1) * 512],
                                start=(k == 0 and not K8),
                                stop=(k == KT - 1))
                        eng = cast_engines[ci % len(cast_engines)]
                        ci += 1
                        dst = ot[:, m * 512:(m + 1) * 512]
                        if eng is nc.scalar:
                            eng.copy(dst, ps[:])
                        else:
                            eng.tensor_copy(dst, ps[:])
                        if last:
                            # per-m output DMAs on the final piece: overlap
                            # DMA with the remaining casts (shorter tail)
                            nc.sync.dma_start(
                                cm.ap()[:, n * OGW + (m0 + m) * 512:
                                           n * OGW + (m0 + m + 1) * 512],
                                dst)
                    if not last:
                        nc.sync.dma_start(
                            cm.ap()[:, n * OGW + m0 * 512:
                                       n * OGW + (m0 + mts) * 512],
                            ot[:])
                m0 += mts
    nc.compile()
    return nc


def _device_matmul(a, bmat, sel=None):
    """a [M_FULL, K0] @ bmat [K0, N_OUT] on 8 cores (rows sharded). Pads K.

    If `sel` (sorted row indices) is given, only those rows are computed on
    device (the LSTM scan ignores t >= length rows); the rest return 0.
    """
    from concourse import bass_utils
    K0 = a.shape[1]
    K = ((K0 + 127) // 128) * 128
    rows = a if sel is None else a[sel]
    S = rows.shape[0]
    tiles = -(-S // 128)
    MT = max(1, -(-tiles // M_WAYS))            # ceil(ceil(S/128)/M_WAYS)
    M_LC = MT * 128
    M_TOT = M_LC * M_WAYS
    NQ = N_OUT // N_WAYS
    a_p = np.zeros((M_TOT, K), np.float32)
    a_p[:S, :K0] = rows
    b_p = np.zeros((K, N_OUT), np.float32)
    b_p[:K0, :] = bmat
    # the first 256 contraction rows run in fp8+DoubleRow (1.44x on that
    # slice); simulated max rel err 1.35e-2 with both layers vs the 2e-2
    # gate (inputs are deterministic, so the sim is faithful)
    K8 = 256 if K >= 384 else 0
    KP = K8 // 256
    if (K, MT) not in _CACHE:
        _CACHE[(K, MT)] = _build_matmul_nc(K, MT, K8)
    nc = _CACHE[(K, MT)]
    KT = (K - K8) // 128
    at_full = np.ascontiguousarray(a_p.T)               # [K, M_TOT] f32
    at16 = at_full[K8:].astype(BF16)
    at_h = [np.ascontiguousarray(at16[:, h * M_LC:(h + 1) * M_LC])
            for h in range(M_WAYS)]
    # quarter q: [K-K8, NQ] -> [128, NCH_LOC, KT, 512]
    b_bf = b_p[K8:].astype(BF16)
    bm_q = [np.ascontiguousarray(
                b_bf[:, q * NQ:(q + 1) * NQ]
                .reshape(KT, 128, NCH_LOC, 512).transpose(1, 2, 0, 3)
            ).reshape(128, NCH_LOC * KT * 512) for q in range(N_WAYS)]
    in_maps = [{"at": at_h[c // N_WAYS], "bm": bm_q[c % N_WAYS]}
               for c in range(N_CORES)]
    if K8:
        at8_full = at_full[:K8].astype(FP8)
        at8_h = [np.ascontiguousarray(at8_full[:, h * M_LC:(h + 1) * M_LC])
                 for h in range(M_WAYS)]
        b8 = b_p[:K8].astype(FP8)
        # [K8, NQ] -> [128, NCH_LOC, KP, 2, 512]
        bm8_q = [np.ascontiguousarray(
                     b8[:, q * NQ:(q + 1) * NQ]
                     .reshape(KP, 2, 128, NCH_LOC, 512)
                     .transpose(2, 3, 0, 1, 4)
                 ).reshape(128, NCH_LOC * KP * 2 * 512)
                 for q in range(N_WAYS)]
        for c in range(N_CORES):
            in_maps[c]["at8"] = at8_h[c // N_WAYS]
            in_maps[c]["bm8"] = bm8_q[c % N_WAYS]
    trace = bool(os.environ.get("KERNEL_TRACE"))
    if trace:
        try:
            _install_ntff_hook()
        except Exception:
            trace = False
    res = bass_utils.run_bass_kernel_spmd(
        nc, in_maps, core_ids=list(range(N_CORES)), trace=trace)
    if res.exec_time_ns is not None:
        _device_matmul.exec_ns += res.exec_time_ns
    # per core: [128, NCH_LOC, MT, 512] -> [M_LC, NQ] block (row-half, col-q)
    out = np.empty((M_TOT, N_OUT), np.float32)
    for c in range(N_CORES):
        blk = (res.results[c]["cm"].reshape(128, NCH_LOC, MT, 512)
               .transpose(2, 0, 1, 3).reshape(M_LC, NQ))
        h, q = c // N_WAYS, c % N_WAYS
        out[h * M_LC:(h + 1) * M_LC, q * NQ:(q + 1) * NQ] = blk
    if sel is None:
        return out[:M_FULL]
    full = np.zeros((M_FULL, N_OUT), np.float32)
    full[sel] = out[:S]
    return full


_device_matmul.exec_ns = 0


def _sigmoid(x):
    return 1.0 / (1.0 + np.exp(-x))


def _lstm_scan(xpart, length, wh, bias, reverse):
    """TF LSTMCell recurrence given precomputed x-part of the gates.

    xpart: [B, T, 4H] = x_t @ Wx  (bias NOT included)
    wh:    [H, 4H] recurrent weights.  Masked-update dynamic_rnn semantics:
    bw direction == descending-t scan with the same (t < length) mask.
    """
    H = HID
    h = np.zeros((B, H), np.float32)
    c = np.zeros((B, H), np.float32)
    out = np.zeros((B, T, H), np.float32)
    wh = np.ascontiguousarray(wh, np.float32)
    bias = bias.astype(np.float32)
    trange = range(T - 1, -1, -1) if reverse else range(T)
    for t in trange:
        z = xpart[:, t] + h @ wh + bias
        i = z[:, 0:H]
        j = z[:, H:2 * H]
        f = z[:, 2 * H:3 * H]
        o = z[:, 3 * H:4 * H]
        c_new = _sigmoid(f + 1.0) * c + _sigmoid(i) * np.tanh(j)
        h_new = _sigmoid(o) * np.tanh(c_new)
        m = (t < length)[:, None]
        c = np.where(m, c_new, c)
        h = np.where(m, h_new, h)
        out[:, t] = np.where(m, h_new, 0.0)
    return out


def kernel(inputs_seq, masks, length, embedding, mask_embedding, transition,
           w_fw0, b_fw0, w_bw0, b_bw0, w_fw1, b_fw1, w_bw1, b_bw1,
           crf_w, crf_b, logits_w, logits_b):
    inputs_seq = np.asarray(inputs_seq)
    masks = np.asarray(masks)
    length = np.asarray(length).reshape(-1).astype(np.int64)
    embedding = np.asarray(embedding, np.float32)
    mask_embedding = np.asarray(mask_embedding, np.float32)
    transition = np.asarray(transition, np.float64)

    d0 = WDIM + MDIM
    # ---- input features (lookup = data prep) -------------------------------
    emb = embedding[inputs_seq]              # [B,T,300]
    memb = mask_embedding[masks]             # [B,T,50]
    xcat = np.concatenate([emb, memb], axis=-1).reshape(M_FULL, d0)

    # rows with t >= length[b] never contribute (scan zeroes/holds them), so
    # only compute x-parts for valid rows
    valid = (np.arange(T)[None, :] < length[:, None]).ravel()
    sel = np.flatnonzero(valid)
    if len(sel) == M_FULL:
        sel = None

    # ---- layer 0 x-part on device (8 cores, rows sharded) ------------------
    wx0 = np.concatenate([np.asarray(w_fw0, np.float32)[:d0],
                          np.asarray(w_bw0, np.float32)[:d0]], axis=1)
    xp0 = _device_matmul(xcat.astype(np.float32), wx0, sel)  # [8192, 4096]
    xp0 = xp0.reshape(B, T, 2, 4 * HID)

    fw0 = _lstm_scan(xp0[:, :, 0], length, np.asarray(w_fw0)[d0:],
                     np.asarray(b_fw0), reverse=False)
    bw0 = _lstm_scan(xp0[:, :, 1], length, np.asarray(w_bw0)[d0:],
                     np.asarray(b_bw0), reverse=True)
    out0 = np.concatenate([fw0, bw0], axis=-1)           # [B,T,1024]

    # ---- layer 1 x-part on device ------------------------------------------
    d1 = 2 * HID
    wx1 = np.concatenate([np.asarray(w_fw1, np.float32)[:d1],
                          np.asarray(w_bw1, np.float32)[:d1]], axis=1)
    xp1 = _device_matmul(out0.reshape(M_FULL, d1), wx1, sel)
    xp1 = xp1.reshape(B, T, 2, 4 * HID)

    fw1 = _lstm_scan(xp1[:, :, 0], length, np.asarray(w_fw1)[d1:],
                     np.asarray(b_fw1), reverse=False)
    bw1 = _lstm_scan(xp1[:, :, 1], length, np.asarray(w_bw1)[d1:],
                     np.asarray(b_bw1), reverse=True)
    out1 = np.concatenate([fw1, bw1], axis=-1)           # [B,T,1024]

    # ---- CRF forward probabilities over 2 tags -----------------------------
    e = out1 @ np.asarray(crf_w, np.float64) + np.asarray(crf_b, np.float64)
    alpha = e[:, 0]                                       # [B,2]
    probs = np.zeros((B, T, 2), np.float64)
    m0 = (length > 0)[:, None]
    probs[:, 0] = np.where(m0, _softmax(alpha), 0.0)
    for t in range(1, T):
        s = alpha[:, :, None] + transition[None]          # [B,2,2]
        mx = s.max(axis=1)
        new = mx + np.log(np.exp(s - mx[:, None]).sum(axis=1)) + e[:, t]
        m = (t < length)[:, None]
        alpha = np.where(m, new, alpha)
        probs[:, t] = np.where(m, _softmax(alpha), 0.0)

    # ---- head --------------------------------------------------------------
    p1 = probs[:, :, -1]                                  # [B,T]
    sv = np.einsum('bt,bth->bh', p1, out1)                # [B,1024]
    logits = sv @ np.asarray(logits_w, np.float64) + np.asarray(
        logits_b, np.float64)
    out = _softmax(logits).reshape(B, 1, NCLASSES)
    return out.astype(np.float32)


def _softmax(x):
    mx = x.max(axis=-1, keepdims=True)
    ex = np.exp(x - mx)
    return ex / ex.sum(axis=-1, keepdims=True)


# revision 42
# speedup vs baseline: 1.2220x; 1.1308x over previous
"""Trainium2 Bass kernel for nn_KerasSaSentTensorflow (BiLSTM+CRF sentiment).

Strategy (data-parallel, per sharding hint):
  - The two large gate-preactivation ("x-part") matmuls of the BiLSTM are
    computed on the 8 NeuronCores, sharded across the batch*time rows:
      layer0: Xcat[8192,350] @ [Wx_fw0 | Wx_bw0][350,4096]
      layer1: out0[8192,1024] @ [Wx_fw1 | Wx_bw1][1024,4096]
    Matmuls run in bf16 (inputs + outputs) with fp32 PSUM accumulation:
    1 cycle/row on the PE array vs 4 for fp32, and half the HBM traffic.
  - The sequential time recurrences (h @ Wh per step, tiny work per step but
    strictly serial), CRF forward pass and the final head run on host.

The bass kernel is a K-accumulating tiled matmul, SPMD over 8 cores with the
M (row) dimension sharded; each core computes a [1024, 4096] slice.
"""
import contextlib
import ctypes
import os
import sys
import types

sys.path.insert(0, '/opt/trn_rl_repo')

import numpy as np
import ml_dtypes

BF16 = np.dtype(ml_dtypes.bfloat16)
FP8 = np.dtype(ml_dtypes.float8_e4m3)

B, T = 32, 256
WDIM, MDIM = 300, 50
HID, NCLASSES = 512, 3
N_CORES = 8
M_FULL = B * T            # 8192 rows (b-major: row = b*T + t)
M_LOC = M_FULL // N_CORES # 1024 rows per core
N_OUT = 4 * HID * 2       # 4096 = fw gates | bw gates
M_WAYS = 2                # row shard ways (cores 0-3 vs 4-7)
N_WAYS = 4                # gate-column shard ways (1024 cols per core)
NCH_LOC = N_OUT // N_WAYS // 512   # 512-col chunks per core
MB = 8                    # m-tiles per JIT-loaded at block
WSCALE = 64.0             # weight pre-scale: lifts fp8e4 weights (~N(0,0.02))
                          # out of the subnormal range; casts divide it out

_CACHE = {}


def _install_ntff_hook():
    """Make bass_utils' trace=True path work under axon: synthesize
    antenv.axon_hooks and register the ctypes NTFF profile hook."""
    name = "antenv.axon_hooks"
    mod = sys.modules.get(name)
    if mod is None:
        mod = types.ModuleType(name)
        mod._hook = None
        mod.set_axon_ntff_profile_hook = lambda h, _m=mod: setattr(_m, "_hook", h)
        mod.get_axon_ntff_profile_hook = lambda _m=mod: _m._hook
        sys.modules[name] = mod
        try:
            import antenv
            antenv.axon_hooks = mod
        except ImportError:
            pass
    if mod._hook is not None:
        return
    lib = ctypes.CDLL("/opt/axon/libaxon_pjrt.so")
    if not hasattr(lib, "axon_start_nrt_profile"):
        return
    lib.axon_start_nrt_profile.argtypes = [ctypes.POINTER(ctypes.c_int64),
                                           ctypes.c_size_t]
    lib.axon_start_nrt_profile.restype = ctypes.c_int64
    lib.axon_stop_nrt_profile.argtypes = [ctypes.c_char_p]
    lib.axon_stop_nrt_profile.restype = ctypes.c_int64

    @contextlib.contextmanager
    def _hook(output_dir, device_ids):
        import jax
        jax.devices()
        if device_ids:
            ids = (ctypes.c_int64 * len(device_ids))(*device_ids)
            rc = lib.axon_start_nrt_profile(ids, len(device_ids))
        else:
            rc = lib.axon_start_nrt_profile(None, 0)
        if rc != 0:
            raise RuntimeError(f"axon_start_nrt_profile rc={rc}")
        try:
            yield
        finally:
            n = lib.axon_stop_nrt_profile(str(output_dir).encode())
            if n < 0:
                raise RuntimeError(f"axon_stop_nrt_profile rc={n}")

    mod.set_axon_ntff_profile_hook(_hook)


def _build_matmul_nc(K, MT, K8=0):
    """out[MT*128, N_OUT/4] = at.T @ b, K contraction (K % 128 == 0).

    If K8 > 0 (multiple of 256), the first K8 contraction rows run as
    fp8e4+DoubleRow pair-matmuls (2 k-tiles per instruction, ~1.44x faster)
    and the remaining K-K8 rows in bf16; the fp8 fraction is kept small to
    bound the accuracy loss.
    """
    import concourse.bacc as bacc
    import concourse.mybir as mybir
    import concourse.tile as tile

    bf16 = mybir.dt.bfloat16
    fp8 = mybir.dt.float8e4
    f32 = mybir.dt.float32
    nc = bacc.Bacc("TRN2", target_bir_lowering=False, debug=False,
                   num_devices=N_CORES)
    KT = (K - K8) // 128     # bf16 k-tiles
    KP = K8 // 256           # fp8 DoubleRow pair-instructions
    M_LC = MT * 128
    # Hybrid sharding: rows split 2 ways (cores 0-3 / 4-7), gate columns
    # split 4 ways (NCH_LOC chunks of 512 per core).  This keeps the global
    # 128-row tile count at its minimum (no per-core ceil-to-8 padding).
    # Layouts chosen for few, large DMAs (Sync-engine dispatch is ~0.7us per
    # dma_start):
    #   at: [K-K8, M_LC] row-major; loaded per m-block just in time
    #   bm: host-permuted [128, NCH_LOC, KT, 512]; resident for whole kernel
    #   cm: device writes [128, NCH_LOC, MT, 512]; host permutes back
    at = nc.dram_tensor("at", [K - K8, M_LC], bf16, kind="ExternalInput")
    bm = nc.dram_tensor("bm", [128, NCH_LOC * KT * 512], bf16,
                        kind="ExternalInput")
    cm = nc.dram_tensor("cm", [128, NCH_LOC * MT * 512], bf16,
                        kind="ExternalOutput")
    if K8:
        # fp8 part: at8 rows are k-subtile-major ([KP, 2, 128, M_LC] in dram);
        # bm8 host layout [128, NCH_LOC, KP, 2, 512]
        at8 = nc.dram_tensor("at8", [K8, M_LC], fp8, kind="ExternalInput")
        bm8 = nc.dram_tensor("bm8", [128, NCH_LOC * KP * 2 * 512], fp8,
                             kind="ExternalInput")
    CHW = KT * 512      # bt chunk width (elements per partition)
    OGW = MT * 512      # out chunk width
    # even split into ceil(MT/MB) blocks — avoids a tiny trailing block
    # (costs ~3us of exposed end-of-stream semaphore stall)
    n_blk = -(-MT // MB)
    base, rem = divmod(MT, n_blk)
    plan = [base + (1 if i < rem else 0) for i in range(n_blk)]
    with tile.TileContext(nc) as tc:
        with tc.tile_pool(name="wp", bufs=1) as wp, \
             tc.tile_pool(name="ab", bufs=2) as apool, \
             tc.tile_pool(name="op", bufs=2) as op, \
             tc.tile_pool(name="pp", bufs=8, space="PSUM") as pp:
            # Warmup matmuls on scratch SBUF: the PE clock (HAM gate) needs
            # ~3.4us of activity to ramp 1.2->2.4 GHz; burn that during the
            # initial DMA wait so real matmuls run at full clock.
            wsc = wp.tile([128, 512], bf16)
            nc.gpsimd.memset(wsc[:], 0)
            for _ in range(6):
                psw = pp.tile([128, 512], f32, tag="ps")
                nc.tensor.matmul(psw[:], wsc[:, :128], wsc[:], start=True,
                                 stop=True)
            # both bt chunks stay resident across all m-blocks
            bt_all = wp.tile([128, NCH_LOC * CHW], bf16)
            if K8:
                # fp8 weights resident: [128, NCH_LOC*KP*2, 512]
                bt8_all = wp.tile([128, NCH_LOC * KP * 2, 512], fp8)
                for i in range(NCH_LOC * KP * 2):
                    nc.sync.dma_start(
                        bt8_all[:, i, :],
                        bm8.ap()[:, i * 512:(i + 1) * 512])
            cast_engines = [nc.vector, nc.scalar]
            ci = 0
            m0 = 0
            for b, mts in enumerate(plan):
                BW = mts * 128
                at_b = apool.tile([128, KT * BW], bf16, tag="at")
                if K8:
                    at8_b = apool.tile([128, KP * 2, BW], fp8, tag="at8")
                    for s in range(KP * 2):
                        nc.sync.dma_start(
                            at8_b[:, s, :],
                            at8.ap()[s * 128:(s + 1) * 128,
                                     m0 * 128:m0 * 128 + BW])
                # Interleave this block's at k-tiles with the bt k-slices
                # (block 0 only) so the first groups can start early.
                for k in range(KT):
                    nc.sync.dma_start(
                        at_b[:, k * BW:(k + 1) * BW],
                        at.ap()[k * 128:(k + 1) * 128,
                                m0 * 128:m0 * 128 + BW])
                    if b == 0:
                        nc.sync.dma_start(
                            bt_all[:, k * 512:(k + 1) * 512],
                            bm.ap()[:, k * 512:(k + 1) * 512])
                if b == 0 and NCH_LOC > 1:
                    nc.sync.dma_start(bt_all[:, CHW:], bm.ap()[:, CHW:])
                last_blk = (b == n_blk - 1)
                for n in range(NCH_LOC):
                    bt = bt_all[:, n * CHW:(n + 1) * CHW]
                    ot = op.tile([128, mts * 512], bf16, tag="ot")
                    last = last_blk and (n == NCH_LOC - 1)
                    for m in range(mts):
                        ps = pp.tile([128, 512], f32, tag="ps")
                        for j in range(KP):
                            nc.tensor.matmul(
                                ps[:],
                                at8_b[:, 2 * j:2 * j + 2,
                                      m * 128:(m + 1) * 128],
                                bt8_all[:, n * KP * 2 + 2 * j:
                                           n * KP * 2 + 2 * j + 2, :],
                                start=(j == 0), stop=False,
                                perf_mode=mybir.MatmulPerfMode.DoubleRow)
                        for k in range(KT):
                            nc.tensor.matmul(
                                ps[:],
                                at_b[:, k * BW + m * 128:
                                        k * BW + (m + 1) * 128],
                                bt[:, k * 512:(k + 1) * 512],
                                start=(k == 0 and not K8),
                                stop=(k == KT - 1))
                        eng = cast_engines[ci % len(cast_engines)]
                        ci += 1
                        dst = ot[:, m * 512:(m + 1) * 512]
                        inv = (1.0 / WSCALE) if K8 else 1.0
                        if eng is nc.scalar:
                            if K8:
                                eng.activation(
                                    dst, ps[:],
                                    mybir.ActivationFunctionType.Copy,
                                    scale=inv)
                            else:
                                eng.copy(dst, ps[:])
                        else:
                            if K8:
                                eng.tensor_scalar_mul(dst, ps[:], inv)
                            else:
                                eng.tensor_copy(dst, ps[:])
                        if last:
                            # per-m output DMAs on the final piece: overlap
                            # DMA with the remaining casts (shorter tail)
                            nc.sync.dma_start(
                                cm.ap()[:, n * OGW + (m0 + m) * 512:
                                           n * OGW + (m0 + m + 1) * 512],
                                dst)
                    if not last:
                        nc.sync.dma_start(
                            cm.ap()[:, n * OGW + m0 * 512:
                                       n * OGW + (m0 + mts) * 512],
                            ot[:])
                m0 += mts
    nc.compile()
    return nc


def _device_matmul(a, bmat, sel=None):
    """a [M_FULL, K0] @ bmat [K0, N_OUT] on 8 cores (rows sharded). Pads K.

    If `sel` (sorted row indices) is given, only those rows are computed on
    device (the LSTM scan ignores t >= length rows); the rest return 0.
    """
    from concourse import bass_utils
    K0 = a.shape[1]
    K = ((K0 + 127) // 128) * 128
    rows = a if sel is None else a[sel]
    S = rows.shape[0]
    tiles = -(-S // 128)
    MT = max(1, -(-tiles // M_WAYS))            # ceil(ceil(S/128)/M_WAYS)
    M_LC = MT * 128
    M_TOT = M_LC * M_WAYS
    NQ = N_OUT // N_WAYS
    a_p = np.zeros((M_TOT, K), np.float32)
    a_p[:S, :K0] = rows
    b_p = np.zeros((K, N_OUT), np.float32)
    b_p[:K0, :] = bmat
    # the first K8 contraction rows run in fp8+DoubleRow (1.44x on that
    # slice) with weights pre-scaled by WSCALE out of the e4m3 subnormal
    # range; simulated max rel err 1.28e-2 (L0 256 + L1 768) vs the 2e-2
    # gate (inputs are deterministic, so the sim is faithful)
    K8 = 768 if K >= 1024 else (256 if K >= 384 else 0)
    KP = K8 // 256
    if (K, MT) not in _CACHE:
        _CACHE[(K, MT)] = _build_matmul_nc(K, MT, K8)
    nc = _CACHE[(K, MT)]
    KT = (K - K8) // 128
    at_full = np.ascontiguousarray(a_p.T)               # [K, M_TOT] f32
    at16 = at_full[K8:].astype(BF16)
    at_h = [np.ascontiguousarray(at16[:, h * M_LC:(h + 1) * M_LC])
            for h in range(M_WAYS)]
    wsc = WSCALE if K8 else 1.0
    # quarter q: [K-K8, NQ] -> [128, NCH_LOC, KT, 512]
    b_bf = (b_p[K8:] * wsc).astype(BF16)
    bm_q = [np.ascontiguousarray(
                b_bf[:, q * NQ:(q + 1) * NQ]
                .reshape(KT, 128, NCH_LOC, 512).transpose(1, 2, 0, 3)
            ).reshape(128, NCH_LOC * KT * 512) for q in range(N_WAYS)]
    in_maps = [{"at": at_h[c // N_WAYS], "bm": bm_q[c % N_WAYS]}
               for c in range(N_CORES)]
    if K8:
        at8_full = at_full[:K8].astype(FP8)
        at8_h = [np.ascontiguousarray(at8_full[:, h * M_LC:(h + 1) * M_LC])
                 for h in range(M_WAYS)]
        b8 = (b_p[:K8] * wsc).astype(FP8)
        # [K8, NQ] -> [128, NCH_LOC, KP, 2, 512]
        bm8_q = [np.ascontiguousarray(
                     b8[:, q * NQ:(q + 1) * NQ]
                     .reshape(KP, 2, 128, NCH_LOC, 512)
                     .transpose(2, 3, 0, 1, 4)
                 ).reshape(128, NCH_LOC * KP * 2 * 512)
                 for q in range(N_WAYS)]
        for c in range(N_CORES):
            in_maps[c]["at8"] = at8_h[c // N_WAYS]
            in_maps[c]["bm8"] = bm8_q[c % N_WAYS]
    trace = bool(os.environ.get("KERNEL_TRACE"))
    if trace:
        try:
            _install_ntff_hook()
        except Exception:
            trace = False
    res = bass_utils.run_bass_kernel_spmd(
        nc, in_maps, core_ids=list(range(N_CORES)), trace=trace)
    if res.exec_time_ns is not None:
        _device_matmul.exec_ns += res.exec_time_ns
    # per core: [128, NCH_LOC, MT, 512] -> [M_LC, NQ] block (row-half, col-q)
    out = np.empty((M_TOT, N_OUT), np.float32)
    for c in range(N_CORES):
        blk = (res.results[c]["cm"].reshape(128, NCH_LOC, MT, 512)
               .transpose(2, 0, 1, 3).reshape(M_LC, NQ))
        h, q = c // N_WAYS, c % N_WAYS
        out[h * M_LC:(h + 1) * M_LC, q * NQ:(q + 1) * NQ] = blk
    if sel is None:
        return out[:M_FULL]
    full = np.zeros((M_FULL, N_OUT), np.float32)
    full[sel] = out[:S]
    return full


_device_matmul.exec_ns = 0


def _sigmoid(x):
    return 1.0 / (1.0 + np.exp(-x))


def _lstm_scan(xpart, length, wh, bias, reverse):
    """TF LSTMCell recurrence given precomputed x-part of the gates.

    xpart: [B, T, 4H] = x_t @ Wx  (bias NOT included)
    wh:    [H, 4H] recurrent weights.  Masked-update dynamic_rnn semantics:
    bw direction == descending-t scan with the same (t < length) mask.
    """
    H = HID
    h = np.zeros((B, H), np.float32)
    c = np.zeros((B, H), np.float32)
    out = np.zeros((B, T, H), np.float32)
    wh = np.ascontiguousarray(wh, np.float32)
    bias = bias.astype(np.float32)
    trange = range(T - 1, -1, -1) if reverse else range(T)
    for t in trange:
        z = xpart[:, t] + h @ wh + bias
        i = z[:, 0:H]
        j = z[:, H:2 * H]
        f = z[:, 2 * H:3 * H]
        o = z[:, 3 * H:4 * H]
        c_new = _sigmoid(f + 1.0) * c + _sigmoid(i) * np.tanh(j)
        h_new = _sigmoid(o) * np.tanh(c_new)
        m = (t < length)[:, None]
        c = np.where(m, c_new, c)
        h = np.where(m, h_new, h)
        out[:, t] = np.where(m, h_new, 0.0)
    return out


def kernel(inputs_seq, masks, length, embedding, mask_embedding, transition,
           w_fw0, b_fw0, w_bw0, b_bw0, w_fw1, b_fw1, w_bw1, b_bw1,
           crf_w, crf_b, logits_w, logits_b):
    inputs_seq = np.asarray(inputs_seq)
    masks = np.asarray(masks)
    length = np.asarray(length).reshape(-1).astype(np.int64)
    embedding = np.asarray(embedding, np.float32)
    mask_embedding = np.asarray(mask_embedding, np.float32)
    transition = np.asarray(transition, np.float64)

    d0 = WDIM + MDIM
    # ---- input features (lookup = data prep) -------------------------------
    emb = embedding[inputs_seq]              # [B,T,300]
    memb = mask_embedding[masks]             # [B,T,50]
    xcat = np.concatenate([emb, memb], axis=-1).reshape(M_FULL, d0)

    # rows with t >= length[b] never contribute (scan zeroes/holds them), so
    # only compute x-parts for valid rows
    valid = (np.arange(T)[None, :] < length[:, None]).ravel()
    sel = np.flatnonzero(valid)
    if len(sel) == M_FULL:
        sel = None

    # ---- layer 0 x-part on device (8 cores, rows sharded) ------------------
    wx0 = np.concatenate([np.asarray(w_fw0, np.float32)[:d0],
                          np.asarray(w_bw0, np.float32)[:d0]], axis=1)
    xp0 = _device_matmul(xcat.astype(np.float32), wx0, sel)  # [8192, 4096]
    xp0 = xp0.reshape(B, T, 2, 4 * HID)

    fw0 = _lstm_scan(xp0[:, :, 0], length, np.asarray(w_fw0)[d0:],
                     np.asarray(b_fw0), reverse=False)
    bw0 = _lstm_scan(xp0[:, :, 1], length, np.asarray(w_bw0)[d0:],
                     np.asarray(b_bw0), reverse=True)
    out0 = np.concatenate([fw0, bw0], axis=-1)           # [B,T,1024]

    # ---- layer 1 x-part on device ------------------------------------------
    d1 = 2 * HID
    wx1 = np.concatenate([np.asarray(w_fw1, np.float32)[:d1],
                          np.asarray(w_bw1, np.float32)[:d1]], axis=1)
    xp1 = _device_matmul(out0.reshape(M_FULL, d1), wx1, sel)
    xp1 = xp1.reshape(B, T, 2, 4 * HID)

    fw1 = _lstm_scan(xp1[:, :, 0], length, np.asarray(w_fw1)[d1:],
                     np.asarray(b_fw1), reverse=False)
    bw1 = _lstm_scan(xp1[:, :, 1], length, np.asarray(w_bw1)[d1:],
                     np.asarray(b_bw1), reverse=True)
    out1 = np.concatenate([fw1, bw1], axis=-1)           # [B,T,1024]

    # ---- CRF forward probabilities over 2 tags -----------------------------
    e = out1 @ np.asarray(crf_w, np.float64) + np.asarray(crf_b, np.float64)
    alpha = e[:, 0]                                       # [B,2]
    probs = np.zeros((B, T, 2), np.float64)
    m0 = (length > 0)[:, None]
    probs[:, 0] = np.where(m0, _softmax(alpha), 0.0)
    for t in range(1, T):
        s = alpha[:, :, None] + transition[None]          # [B,2,2]
        mx = s.max(axis=1)
        new = mx + np.log(np.exp(s - mx[:, None]).sum(axis=1)) + e[:, t]
        m = (t < length)[:, None]
        alpha = np.where(m, new, alpha)
        probs[:, t] = np.where(m, _softmax(alpha), 0.0)

    # ---- head --------------------------------------------------------------
    p1 = probs[:, :, -1]                                  # [B,T]
    sv = np.einsum('bt,bth->bh', p1, out1)                # [B,1024]
    logits = sv @ np.asarray(logits_w, np.float64) + np.asarray(
        logits_b, np.float64)
    out = _softmax(logits).reshape(B, 1, NCLASSES)
    return out.astype(np.float32)


def _softmax(x):
    mx = x.max(axis=-1, keepdims=True)
    ex = np.exp(x - mx)
    return ex / ex.sum(axis=-1, keepdims=True)


# revision 46
# speedup vs baseline: 1.2994x; 1.0634x over previous
"""Trainium2 Bass kernel for nn_KerasSaSentTensorflow (BiLSTM+CRF sentiment).

Strategy (data-parallel, per sharding hint):
  - The two large gate-preactivation ("x-part") matmuls of the BiLSTM are
    computed on the 8 NeuronCores, sharded across the batch*time rows:
      layer0: Xcat[8192,350] @ [Wx_fw0 | Wx_bw0][350,4096]
      layer1: out0[8192,1024] @ [Wx_fw1 | Wx_bw1][1024,4096]
    Matmuls run in bf16 (inputs + outputs) with fp32 PSUM accumulation:
    1 cycle/row on the PE array vs 4 for fp32, and half the HBM traffic.
  - The sequential time recurrences (h @ Wh per step, tiny work per step but
    strictly serial), CRF forward pass and the final head run on host.

The bass kernel is a K-accumulating tiled matmul, SPMD over 8 cores with the
M (row) dimension sharded; each core computes a [1024, 4096] slice.
"""
import contextlib
import ctypes
import os
import sys
import types

sys.path.insert(0, '/opt/trn_rl_repo')

import numpy as np
import ml_dtypes

BF16 = np.dtype(ml_dtypes.bfloat16)
FP8 = np.dtype(ml_dtypes.float8_e4m3)

B, T = 32, 256
WDIM, MDIM = 300, 50
HID, NCLASSES = 512, 3
N_CORES = 8
M_FULL = B * T            # 8192 rows (b-major: row = b*T + t)
M_LOC = M_FULL // N_CORES # 1024 rows per core
N_OUT = 4 * HID * 2       # 4096 = fw gates | bw gates
M_WAYS = 2                # row shard ways (cores 0-3 vs 4-7)
N_WAYS = 4                # gate-column shard ways (1024 cols per core)
NCH_LOC = N_OUT // N_WAYS // 512   # 512-col chunks per core
MB = 8                    # m-tiles per JIT-loaded at block
WSCALE = 64.0             # weight pre-scale: lifts fp8e4 weights (~N(0,0.02))
                          # out of the subnormal range; casts divide it out

_CACHE = {}


def _install_ntff_hook():
    """Make bass_utils' trace=True path work under axon: synthesize
    antenv.axon_hooks and register the ctypes NTFF profile hook."""
    name = "antenv.axon_hooks"
    mod = sys.modules.get(name)
    if mod is None:
        mod = types.ModuleType(name)
        mod._hook = None
        mod.set_axon_ntff_profile_hook = lambda h, _m=mod: setattr(_m, "_hook", h)
        mod.get_axon_ntff_profile_hook = lambda _m=mod: _m._hook
        sys.modules[name] = mod
        try:
            import antenv
            antenv.axon_hooks = mod
        except ImportError:
            pass
    if mod._hook is not None:
        return
    lib = ctypes.CDLL("/opt/axon/libaxon_pjrt.so")
    if not hasattr(lib, "axon_start_nrt_profile"):
        return
    lib.axon_start_nrt_profile.argtypes = [ctypes.POINTER(ctypes.c_int64),
                                           ctypes.c_size_t]
    lib.axon_start_nrt_profile.restype = ctypes.c_int64
    lib.axon_stop_nrt_profile.argtypes = [ctypes.c_char_p]
    lib.axon_stop_nrt_profile.restype = ctypes.c_int64

    @contextlib.contextmanager
    def _hook(output_dir, device_ids):
        import jax
        jax.devices()
        if device_ids:
            ids = (ctypes.c_int64 * len(device_ids))(*device_ids)
            rc = lib.axon_start_nrt_profile(ids, len(device_ids))
        else:
            rc = lib.axon_start_nrt_profile(None, 0)
        if rc != 0:
            raise RuntimeError(f"axon_start_nrt_profile rc={rc}")
        try:
            yield
        finally:
            n = lib.axon_stop_nrt_profile(str(output_dir).encode())
            if n < 0:
                raise RuntimeError(f"axon_stop_nrt_profile rc={n}")

    mod.set_axon_ntff_profile_hook(_hook)


def _build_matmul_nc(K, MT, K8=0):
    """out[MT*128, N_OUT/4] = at.T @ b, K contraction (K % 128 == 0).

    If K8 > 0 (multiple of 256), the first K8 contraction rows run as
    fp8e4+DoubleRow pair-matmuls (2 k-tiles per instruction, ~1.44x faster)
    and the remaining K-K8 rows in bf16; the fp8 fraction is kept small to
    bound the accuracy loss.
    """
    import concourse.bacc as bacc
    import concourse.mybir as mybir
    import concourse.tile as tile

    bf16 = mybir.dt.bfloat16
    fp8 = mybir.dt.float8e4
    f32 = mybir.dt.float32
    nc = bacc.Bacc("TRN2", target_bir_lowering=False, debug=False,
                   num_devices=N_CORES)
    KT = (K - K8) // 128     # bf16 k-tiles
    KP = K8 // 256           # fp8 DoubleRow pair-instructions
    M_LC = MT * 128
    # Hybrid sharding: rows split 2 ways (cores 0-3 / 4-7), gate columns
    # split 4 ways (NCH_LOC chunks of 512 per core).  This keeps the global
    # 128-row tile count at its minimum (no per-core ceil-to-8 padding).
    # Layouts chosen for few, large DMAs (Sync-engine dispatch is ~0.7us per
    # dma_start):
    #   at: [K-K8, M_LC] row-major; loaded per m-block just in time
    #   bm: host-permuted [128, NCH_LOC, KT, 512]; resident for whole kernel
    #   cm: device writes [128, NCH_LOC, MT, 512]; host permutes back
    at = nc.dram_tensor("at", [K - K8, M_LC], bf16, kind="ExternalInput")
    bm = nc.dram_tensor("bm", [128, NCH_LOC * KT * 512], bf16,
                        kind="ExternalInput")
    cm = nc.dram_tensor("cm", [128, NCH_LOC * MT * 512], bf16,
                        kind="ExternalOutput")
    if K8:
        # fp8 part, partition-major 3D layouts so each loads in ONE DMA
        # (the Sync engine pays ~0.6us per dispatch)
        at8 = nc.dram_tensor("at8", [128, KP * 2, M_LC], fp8,
                             kind="ExternalInput")
        bm8 = nc.dram_tensor("bm8", [128, NCH_LOC * KP * 2, 512], fp8,
                             kind="ExternalInput")
    CHW = KT * 512      # bt chunk width (elements per partition)
    OGW = MT * 512      # out chunk width
    # even split into ceil(MT/MB) blocks — avoids a tiny trailing block
    # (costs ~3us of exposed end-of-stream semaphore stall)
    n_blk = -(-MT // MB)
    base, rem = divmod(MT, n_blk)
    plan = [base + (1 if i < rem else 0) for i in range(n_blk)]
    with tile.TileContext(nc) as tc:
        with tc.tile_pool(name="wp", bufs=1) as wp, \
             tc.tile_pool(name="ab", bufs=2) as apool, \
             tc.tile_pool(name="op", bufs=2) as op, \
             tc.tile_pool(name="pp", bufs=8, space="PSUM") as pp:
            # Warmup matmuls on scratch SBUF: the PE clock (HAM gate) needs
            # ~3.4us of activity to ramp 1.2->2.4 GHz; burn that during the
            # initial DMA wait so real matmuls run at full clock.
            wsc = wp.tile([128, 512], bf16)
            nc.gpsimd.memset(wsc[:], 0)
            for _ in range(6):
                psw = pp.tile([128, 512], f32, tag="ps")
                nc.tensor.matmul(psw[:], wsc[:, :128], wsc[:], start=True,
                                 stop=True)
            # both bt chunks stay resident across all m-blocks
            bt_all = wp.tile([128, NCH_LOC * CHW], bf16)
            if K8:
                # fp8 weights resident: [128, NCH_LOC*KP*2, 512], one DMA
                bt8_all = wp.tile([128, NCH_LOC * KP * 2, 512], fp8)
                nc.sync.dma_start(bt8_all[:], bm8.ap())
            cast_engines = [nc.vector, nc.scalar]
            ci = 0
            m0 = 0
            for b, mts in enumerate(plan):
                BW = mts * 128
                at_b = apool.tile([128, KT * BW], bf16, tag="at")
                if K8:
                    at8_b = apool.tile([128, KP * 2, BW], fp8, tag="at8")
                    nc.sync.dma_start(
                        at8_b[:],
                        at8.ap()[:, :, m0 * 128:m0 * 128 + BW])
                # Interleave this block's at k-tiles with the bt k-slices
                # (block 0 only) so the first groups can start early.
                for k in range(KT):
                    nc.sync.dma_start(
                        at_b[:, k * BW:(k + 1) * BW],
                        at.ap()[k * 128:(k + 1) * 128,
                                m0 * 128:m0 * 128 + BW])
                    if b == 0:
                        nc.sync.dma_start(
                            bt_all[:, k * 512:(k + 1) * 512],
                            bm.ap()[:, k * 512:(k + 1) * 512])
                if b == 0 and NCH_LOC > 1:
                    nc.sync.dma_start(bt_all[:, CHW:], bm.ap()[:, CHW:])
                last_blk = (b == n_blk - 1)
                for n in range(NCH_LOC):
                    bt = bt_all[:, n * CHW:(n + 1) * CHW]
                    ot = op.tile([128, mts * 512], bf16, tag="ot")
                    last = last_blk and (n == NCH_LOC - 1)
                    for m in range(mts):
                        ps = pp.tile([128, 512], f32, tag="ps")
                        for j in range(KP):
                            nc.tensor.matmul(
                                ps[:],
                                at8_b[:, 2 * j:2 * j + 2,
                                      m * 128:(m + 1) * 128],
                                bt8_all[:, n * KP * 2 + 2 * j:
                                           n * KP * 2 + 2 * j + 2, :],
                                start=(j == 0), stop=False,
                                perf_mode=mybir.MatmulPerfMode.DoubleRow)
                        for k in range(KT):
                            nc.tensor.matmul(
                                ps[:],
                                at_b[:, k * BW + m * 128:
                                        k * BW + (m + 1) * 128],
                                bt[:, k * 512:(k + 1) * 512],
                                start=(k == 0 and not K8),
                                stop=(k == KT - 1))
                        eng = cast_engines[ci % len(cast_engines)]
                        ci += 1
                        dst = ot[:, m * 512:(m + 1) * 512]
                        inv = (1.0 / WSCALE) if K8 else 1.0
                        if eng is nc.scalar:
                            if K8:
                                eng.activation(
                                    dst, ps[:],
                                    mybir.ActivationFunctionType.Copy,
                                    scale=inv)
                            else:
                                eng.copy(dst, ps[:])
                        else:
                            if K8:
                                eng.tensor_scalar_mul(dst, ps[:], inv)
                            else:
                                eng.tensor_copy(dst, ps[:])
                        if last:
                            # per-m output DMAs on the final piece: overlap
                            # DMA with the remaining casts (shorter tail)
                            nc.sync.dma_start(
                                cm.ap()[:, n * OGW + (m0 + m) * 512:
                                           n * OGW + (m0 + m + 1) * 512],
                                dst)
                    if not last:
                        nc.sync.dma_start(
                            cm.ap()[:, n * OGW + m0 * 512:
                                       n * OGW + (m0 + mts) * 512],
                            ot[:])
                m0 += mts
    nc.compile()
    return nc


def _device_matmul(a, bmat, sel=None):
    """a [M_FULL, K0] @ bmat [K0, N_OUT] on 8 cores (rows sharded). Pads K.

    If `sel` (sorted row indices) is given, only those rows are computed on
    device (the LSTM scan ignores t >= length rows); the rest return 0.
    """
    from concourse import bass_utils
    K0 = a.shape[1]
    K = ((K0 + 127) // 128) * 128
    rows = a if sel is None else a[sel]
    S = rows.shape[0]
    tiles = -(-S // 128)
    MT = max(1, -(-tiles // M_WAYS))            # ceil(ceil(S/128)/M_WAYS)
    M_LC = MT * 128
    M_TOT = M_LC * M_WAYS
    NQ = N_OUT // N_WAYS
    a_p = np.zeros((M_TOT, K), np.float32)
    a_p[:S, :K0] = rows
    b_p = np.zeros((K, N_OUT), np.float32)
    b_p[:K0, :] = bmat
    # the first K8 contraction rows run in fp8+DoubleRow (1.44x on that
    # slice) with weights pre-scaled by WSCALE out of the e4m3 subnormal
    # range; simulated max rel err 1.28e-2 (L0 256 + L1 768) vs the 2e-2
    # gate (inputs are deterministic, so the sim is faithful)
    K8 = 768 if K >= 1024 else (256 if K >= 384 else 0)
    KP = K8 // 256
    if (K, MT) not in _CACHE:
        _CACHE[(K, MT)] = _build_matmul_nc(K, MT, K8)
    nc = _CACHE[(K, MT)]
    KT = (K - K8) // 128
    at_full = np.ascontiguousarray(a_p.T)               # [K, M_TOT] f32
    at16 = at_full[K8:].astype(BF16)
    at_h = [np.ascontiguousarray(at16[:, h * M_LC:(h + 1) * M_LC])
            for h in range(M_WAYS)]
    wsc = WSCALE if K8 else 1.0
    # quarter q: [K-K8, NQ] -> [128, NCH_LOC, KT, 512]
    b_bf = (b_p[K8:] * wsc).astype(BF16)
    bm_q = [np.ascontiguousarray(
                b_bf[:, q * NQ:(q + 1) * NQ]
                .reshape(KT, 128, NCH_LOC, 512).transpose(1, 2, 0, 3)
            ).reshape(128, NCH_LOC * KT * 512) for q in range(N_WAYS)]
    in_maps = [{"at": at_h[c // N_WAYS], "bm": bm_q[c % N_WAYS]}
               for c in range(N_CORES)]
    if K8:
        # [K8, M_TOT] -> partition-major [128, KP*2, M_TOT]
        at8_full = (at_full[:K8].astype(FP8)
                    .reshape(KP * 2, 128, M_TOT).transpose(1, 0, 2))
        at8_h = [np.ascontiguousarray(at8_full[:, :, h * M_LC:(h + 1) * M_LC])
                 for h in range(M_WAYS)]
        b8 = (b_p[:K8] * wsc).astype(FP8)
        # [K8, NQ] -> [128, NCH_LOC*KP*2, 512]
        bm8_q = [np.ascontiguousarray(
                     b8[:, q * NQ:(q + 1) * NQ]
                     .reshape(KP, 2, 128, NCH_LOC, 512)
                     .transpose(2, 3, 0, 1, 4)
                 ).reshape(128, NCH_LOC * KP * 2, 512)
                 for q in range(N_WAYS)]
        for c in range(N_CORES):
            in_maps[c]["at8"] = at8_h[c // N_WAYS]
            in_maps[c]["bm8"] = bm8_q[c % N_WAYS]
    trace = bool(os.environ.get("KERNEL_TRACE"))
    if trace:
        try:
            _install_ntff_hook()
        except Exception:
            trace = False
    res = bass_utils.run_bass_kernel_spmd(
        nc, in_maps, core_ids=list(range(N_CORES)), trace=trace)
    if res.exec_time_ns is not None:
        _device_matmul.exec_ns += res.exec_time_ns
    # per core: [128, NCH_LOC, MT, 512] -> [M_LC, NQ] block (row-half, col-q)
    out = np.empty((M_TOT, N_OUT), np.float32)
    for c in range(N_CORES):
        blk = (res.results[c]["cm"].reshape(128, NCH_LOC, MT, 512)
               .transpose(2, 0, 1, 3).reshape(M_LC, NQ))
        h, q = c // N_WAYS, c % N_WAYS
        out[h * M_LC:(h + 1) * M_LC, q * NQ:(q + 1) * NQ] = blk
    if sel is None:
        return out[:M_FULL]
    full = np.zeros((M_FULL, N_OUT), np.float32)
    full[sel] = out[:S]
    return full


_device_matmul.exec_ns = 0


def _sigmoid(x):
    return 1.0 / (1.0 + np.exp(-x))


def _lstm_scan(xpart, length, wh, bias, reverse):
    """TF LSTMCell recurrence given precomputed x-part of the gates.

    xpart: [B, T, 4H] = x_t @ Wx  (bias NOT included)
    wh:    [H, 4H] recurrent weights.  Masked-update dynamic_rnn semantics:
    bw direction == descending-t scan with the same (t < length) mask.
    """
    H = HID
    h = np.zeros((B, H), np.float32)
    c = np.zeros((B, H), np.float32)
    out = np.zeros((B, T, H), np.float32)
    wh = np.ascontiguousarray(wh, np.float32)
    bias = bias.astype(np.float32)
    trange = range(T - 1, -1, -1) if reverse else range(T)
    for t in trange:
        z = xpart[:, t] + h @ wh + bias
        i = z[:, 0:H]
        j = z[:, H:2 * H]
        f = z[:, 2 * H:3 * H]
        o = z[:, 3 * H:4 * H]
        c_new = _sigmoid(f + 1.0) * c + _sigmoid(i) * np.tanh(j)
        h_new = _sigmoid(o) * np.tanh(c_new)
        m = (t < length)[:, None]
        c = np.where(m, c_new, c)
        h = np.where(m, h_new, h)
        out[:, t] = np.where(m, h_new, 0.0)
    return out


def kernel(inputs_seq, masks, length, embedding, mask_embedding, transition,
           w_fw0, b_fw0, w_bw0, b_bw0, w_fw1, b_fw1, w_bw1, b_bw1,
           crf_w, crf_b, logits_w, logits_b):
    inputs_seq = np.asarray(inputs_seq)
    masks = np.asarray(masks)
    length = np.asarray(length).reshape(-1).astype(np.int64)
    embedding = np.asarray(embedding, np.float32)
    mask_embedding = np.asarray(mask_embedding, np.float32)
    transition = np.asarray(transition, np.float64)

    d0 = WDIM + MDIM
    # ---- input features (lookup = data prep) -------------------------------
    emb = embedding[inputs_seq]              # [B,T,300]
    memb = mask_embedding[masks]             # [B,T,50]
    xcat = np.concatenate([emb, memb], axis=-1).reshape(M_FULL, d0)

    # rows with t >= length[b] never contribute (scan zeroes/holds them), so
    # only compute x-parts for valid rows
    valid = (np.arange(T)[None, :] < length[:, None]).ravel()
    sel = np.flatnonzero(valid)
    if len(sel) == M_FULL:
        sel = None

    # ---- layer 0 x-part on device (8 cores, rows sharded) ------------------
    wx0 = np.concatenate([np.asarray(w_fw0, np.float32)[:d0],
                          np.asarray(w_bw0, np.float32)[:d0]], axis=1)
    xp0 = _device_matmul(xcat.astype(np.float32), wx0, sel)  # [8192, 4096]
    xp0 = xp0.reshape(B, T, 2, 4 * HID)

    fw0 = _lstm_scan(xp0[:, :, 0], length, np.asarray(w_fw0)[d0:],
                     np.asarray(b_fw0), reverse=False)
    bw0 = _lstm_scan(xp0[:, :, 1], length, np.asarray(w_bw0)[d0:],
                     np.asarray(b_bw0), reverse=True)
    out0 = np.concatenate([fw0, bw0], axis=-1)           # [B,T,1024]

    # ---- layer 1 x-part on device ------------------------------------------
    d1 = 2 * HID
    wx1 = np.concatenate([np.asarray(w_fw1, np.float32)[:d1],
                          np.asarray(w_bw1, np.float32)[:d1]], axis=1)
    xp1 = _device_matmul(out0.reshape(M_FULL, d1), wx1, sel)
    xp1 = xp1.reshape(B, T, 2, 4 * HID)

    fw1 = _lstm_scan(xp1[:, :, 0], length, np.asarray(w_fw1)[d1:],
                     np.asarray(b_fw1), reverse=False)
    bw1 = _lstm_scan(xp1[:, :, 1], length, np.asarray(w_bw1)[d1:],
                     np.asarray(b_bw1), reverse=True)
    out1 = np.concatenate([fw1, bw1], axis=-1)           # [B,T,1024]

    # ---- CRF forward probabilities over 2 tags -----------------------------
    e = out1 @ np.asarray(crf_w, np.float64) + np.asarray(crf_b, np.float64)
    alpha = e[:, 0]                                       # [B,2]
    probs = np.zeros((B, T, 2), np.float64)
    m0 = (length > 0)[:, None]
    probs[:, 0] = np.where(m0, _softmax(alpha), 0.0)
    for t in range(1, T):
        s = alpha[:, :, None] + transition[None]          # [B,2,2]
        mx = s.max(axis=1)
        new = mx + np.log(np.exp(s - mx[:, None]).sum(axis=1)) + e[:, t]
        m = (t < length)[:, None]
        alpha = np.where(m, new, alpha)
        probs[:, t] = np.where(m, _softmax(alpha), 0.0)

    # ---- head --------------------------------------------------------------
    p1 = probs[:, :, -1]                                  # [B,T]
    sv = np.einsum('bt,bth->bh', p1, out1)                # [B,1024]
    logits = sv @ np.asarray(logits_w, np.float64) + np.asarray(
        logits_b, np.float64)
    out = _softmax(logits).reshape(B, 1, NCLASSES)
    return out.astype(np.float32)


def _softmax(x):
    mx = x.max(axis=-1, keepdims=True)
    ex = np.exp(x - mx)
    return ex / ex.sum(axis=-1, keepdims=True)
